# revision 45
# baseline (speedup 1.0000x reference)
"""Causal self-attention on 8 trn2 NeuronCores.

Sharding: tensor-parallel over heads (2 heads/core) for QKV+attention.  Row
ownership for the output projection is window-interleaved: window w = global
rows [1024w, 1024w+1024) and core j owns rows 1024w+128j..+128 of every
window.  This makes the head-split -> row-split reshard expressible as FOUR
per-window 8-rank AllToAlls that are issued as soon as each strip-pair's
attention completes, hiding most of the collective cost under attention
compute (windows 1+2 are merged into one collective; the last window's
collective is small and constant-overhead-dominated).

Softmax normalization moves to the RECEIVE side of the collective: each core
ships unnormalized attn-out rows plus per-(head,row) reciprocal rows, and the
receiver scales the gathered [1024 dims x 128 rows] tile once before the
output projection.

All matmuls run in bf16 with f32 PSUM accumulation.  Attention is computed in
"scores transposed" layout ([keys, queries] on chip); softmax denominators
come from a ones column appended to V, and the causal mask is a
multiplicative {0,1} bf16 mask (generated on-chip via affine_select) applied
after exp (safe: scores are O(6), no overflow without max-subtraction).
"""

import numpy as np
import ml_dtypes

B, T, D, H, HD = 2, 2048, 1024, 16, 64
NCORES = 8
R = B * T              # 4096 global rows (b*T + t)
HPC = H // NCORES      # 2 heads per core
HDIM = HPC * HD        # 128 dims per core
ROWS_PER_CORE = R // NCORES  # 512
NKT = D // 128         # 8 contraction tiles
NSTRIP = T // 512      # 4 query strips per batch
WPS = 130              # rows per (window, shard) in cc buffers: 128 dims + 2 rec

_BF16 = ml_dtypes.bfloat16
_cache = {}


def _patch_tile_drain():
    """This walrus build rejects >1 sync wait on SP CTRL instructions; split
    the Tile tail-drain waits across single-wait nops."""
    import concourse.mybir as mybir
    import concourse.tile as tile_mod
    from concourse.vector_clock import ScopedClock

    if getattr(tile_mod.TileContext, "_drain_patched", False):
        return

    def _drain_and_barrier(self, tick_clock, wait_clock):
        nc = self.nc
        dummy = mybir.InstNoOp(
            name=nc.get_next_instruction_name(),
            engine=mybir.EngineType.SP,
            ins=[],
            outs=[],
        )
        wait_clock.add_sem_waits(dummy, ScopedClock({None: tick_clock.global_clock}))
        waits = list(dummy.sync_info.on_wait) if dummy.sync_info else []
        for i in range(len(waits)):
            w = nc.sync.nop(nofuse=True, hint="tail_drain_wait")
            w.ins.sync_info = mybir.SyncInfo(on_wait=waits[i : i + 1], on_update=[])
        nc.sync.drain()
        nc.all_engine_barrier()
        assert self.sems is not None
        popped = nc._tile_sem_poison_stack.pop()
        assert popped is self._sem_poison
        nc.clear_and_free_semaphores(list(self.sems.allocated().values()))
        nc.all_engine_barrier()

    tile_mod.TileContext._drain_and_barrier = _drain_and_barrier

    # Body instructions can also accumulate >2 waits (CTRL structs take 1,
    # other structs 2 on this walrus).  Before lowering, move excess waits
    # onto single-wait nops inserted just before the instruction on the same
    # engine stream.
    _orig_lower = tile_mod.TileContext._lower_ordered_insts

    def _lower_split_waits(self, ordered):
        nc = self.nc
        for bb_name, insts in ordered.items():
            new_insts = []
            for inst in insts:
                si = getattr(inst, "sync_info", None)
                waits = list(si.on_wait) if si is not None and si.on_wait else []
                limit = 1
                if len(waits) > limit and inst.engine is not None:
                    keep = waits[: limit - 1] if limit > 1 else []
                    spill = waits[len(keep) :][:-1]
                    keep = keep + [waits[-1]]
                    for w in spill:
                        nop = mybir.InstNoOp(
                            name=nc.get_next_instruction_name(),
                            engine=inst.engine,
                            ins=[],
                            outs=[],
                        )
                        nop.sync_info = mybir.SyncInfo(on_wait=[w], on_update=[])
                        nop.debug = inst.debug
                        new_insts.append(nop)
                    inst.sync_info = mybir.SyncInfo(
                        on_wait=keep, on_update=list(si.on_update or [])
                    )
                new_insts.append(inst)
            ordered[bb_name] = new_insts
        return _orig_lower(self, ordered)

    tile_mod.TileContext._lower_ordered_insts = _lower_split_waits
    tile_mod.TileContext._drain_patched = True


def _build():
    import concourse.bass as bass
    import concourse.mybir as mybir
    import concourse.tile as tile
    from concourse.tile import add_dep_helper
    from concourse.masks import make_identity

    _patch_tile_drain()
    f32 = mybir.dt.float32
    bf16 = mybir.dt.bfloat16

    nc = bass.Bass("TRN2", target_bir_lowering=False, debug=False, num_devices=NCORES)

    # ---- DRAM I/O (per core) ----
    xT = nc.dram_tensor("xT", [D, R], bf16, kind="ExternalInput").ap()
    wqT = nc.dram_tensor("wqT", [D, HDIM], bf16, kind="ExternalInput").ap()
    wkT = nc.dram_tensor("wkT", [D, HDIM], bf16, kind="ExternalInput").ap()
    wvT = nc.dram_tensor("wvT", [D, HDIM], bf16, kind="ExternalInput").ap()
    bq_s = nc.dram_tensor("bq_s", [HDIM, 1], f32, kind="ExternalInput").ap()
    bk_s = nc.dram_tensor("bk_s", [HDIM, 1], f32, kind="ExternalInput").ap()
    bv_s = nc.dram_tensor("bv_s", [HDIM, 1], f32, kind="ExternalInput").ap()
    woT = nc.dram_tensor("woT", [D, D], bf16, kind="ExternalInput").ap()
    bo_row = nc.dram_tensor("bo_row", [1, D], bf16, kind="ExternalInput").ap()
    out = nc.dram_tensor("out", [ROWS_PER_CORE, D], f32, kind="ExternalOutput").ap()

    # collective bounce buffers. windows: w = 2b + s0//2 covers global rows
    # [1024w, 1024w+1024); shard j = core j's 128 owned rows of the window.
    # shard layout (WPS=130 rows): 0:128 = 2-head dims, 128:130 = reciprocals.
    # groups: A=[w0], B=[w1,w2] (shard = w1 130 rows then w2 130 rows), C=[w3].
    ccA_in = nc.dram_tensor("ccA_in", [NCORES * WPS, 128], bf16).ap()
    ccA_out = nc.dram_tensor("ccA_out", [NCORES * WPS, 128], bf16).ap()
    ccB_in = nc.dram_tensor("ccB_in", [NCORES * 2 * WPS, 128], bf16).ap()
    ccB_out = nc.dram_tensor("ccB_out", [NCORES * 2 * WPS, 128], bf16).ap()
    ccC_in = nc.dram_tensor("ccC_in", [NCORES * WPS, 128], bf16).ap()
    ccC_out = nc.dram_tensor("ccC_out", [NCORES * WPS, 128], bf16).ap()
    # per-window: (in_ap, out_ap, row offset inside shard, shard stride rows)
    wininfo = {
        0: (ccA_in, ccA_out, 0, WPS),
        1: (ccB_in, ccB_out, 0, 2 * WPS),
        2: (ccB_in, ccB_out, WPS, 2 * WPS),
        3: (ccC_in, ccC_out, 0, WPS),
    }
    win_cc_tag = {0: "A", 1: "B", 2: "B", 3: "C"}

    with tile.TileContext(nc) as tc:
        import contextlib
        import concourse.bass as _bass

        with contextlib.ExitStack() as ctx:
            singles = ctx.enter_context(tc.tile_pool(name="singles", bufs=1))

            # ---- weights on fast queues; x chunks fill the two HWDGE queues
            # (the gpsimd SWDGE queue pays ~1us of Pool desc-gen per DMA, so
            # it only carries the non-urgent loads: wq/biases/bo/wo).
            w_sb = {}
            for (name, src), eng in zip(
                (("v", wvT), ("q", wqT), ("k", wkT)),
                (nc.sync, nc.gpsimd, nc.scalar),
            ):
                t = singles.tile([128, NKT, HDIM], bf16, tag=f"w{name}", name=f"w{name}")
                eng.dma_start(out=t, in_=src.rearrange("(k p) c -> p k c", p=128))
                w_sb[name] = t
            bias_sb = {}
            for name, src in (("q", bq_s), ("k", bk_s), ("v", bv_s)):
                t = singles.tile([HDIM, 1], f32, tag=f"b{name}", name=f"b{name}")
                nc.gpsimd.dma_start(out=t, in_=src)
                bias_sb[name] = t
            # x in 32 chunks [128, 1024], quarter-major k-minor so quarter 0's
            # contraction tiles land first and QKV starts ~5us in.
            feed_engs = [nc.sync, nc.gpsimd, nc.scalar]
            xc_sb = [[None] * NKT for _ in range(4)]
            for qi in range(4):
                for k in range(NKT):
                    t = singles.tile([128, 1024], bf16, tag=f"xc{qi}_{k}")
                    if qi == 0:
                        # quarter 0 gates the first matmuls: keep it off the
                        # slow gpsimd SWDGE queue
                        eng = [nc.sync, nc.scalar][k % 2]
                    else:
                        eng = feed_engs[(NKT * qi + k) % 3]
                    eng.dma_start(
                        out=t,
                        in_=xT[128 * k : 128 * (k + 1), 1024 * qi : 1024 * (qi + 1)],
                    )
                    xc_sb[qi][k] = t
            bo_sb = singles.tile([1, D], bf16, tag="bo")
            nc.gpsimd.dma_start(out=bo_sb, in_=bo_row)
            wo_sb = singles.tile([128, NKT, D], bf16, tag="wo")
            nc.sync.dma_start(out=wo_sb, in_=woT.rearrange("(k p) c -> p k c", p=128))

            # ---- on-chip constants ----
            ident = singles.tile([128, 128], bf16, tag="ident")
            make_identity(nc, ident)
            ones_row = singles.tile([1, 128], bf16, tag="ones")
            nc.vector.memset(ones_row, 1.0)
            # causal masks: block m is [mask_m(512) | ones(512)];
            # mask_m[r, c] = 1.0 iff c - r - 128m >= 0.
            mask_sb = singles.tile([128, 4 * 1024], bf16, tag="mask")
            nc.gpsimd.memset(mask_sb, 1.0)
            mones = singles.tile([128, 512], bf16, tag="mones")
            nc.gpsimd.memset(mones, 1.0)
            for m in range(4):
                nc.gpsimd.affine_select(
                    out=mask_sb[:, 1024 * m : 1024 * m + 512],
                    in_=mones,
                    pattern=[[1, 512]],
                    compare_op=mybir.AluOpType.is_ge,
                    fill=0.0,
                    base=-128 * m,
                    channel_multiplier=-1,
                )
            # preload the Exp activation table off the critical path
            exp_warm = singles.tile([1, 128], bf16, tag="expwarm")
            nc.scalar.activation(
                out=exp_warm,
                in_=ones_row,
                func=mybir.ActivationFunctionType.Exp,
                scale=1.0,
            )

            # persistent activations
            qT_sb = singles.tile([128, R], bf16, tag="qT")   # rows 0-63 head A dims
            kT_sb = singles.tile([128, R], bf16, tag="kT")
            vT_sb = singles.tile([128, R], bf16, tag="vT")
            # v in [keys, dims] layout per key block kb:
            #   cols 0:64 = vA, 64 = ones, 65:129 = vB, 129 = ones
            # so lhsT for head h = cols [65h, 65h+65) = [v_h | ones]:
            # attnout at psum rows 0-63, softmax sums at row 64.
            v_ab = singles.tile([128, R // 128, 130], bf16, tag="vab")
            nc.vector.memset(v_ab[:, :, 64:65], 1.0)
            nc.vector.memset(v_ab[:, :, 129:130], 1.0)

            # ---- phase 1: projections, quarter-by-quarter so compute starts
            # as soon as the first 1MiB of x lands.  V first per quarter so
            # its transposes stay off the critical path.  Emitted in two
            # batch-halves so pair-1 attention (which only needs batch 0)
            # interleaves between them and its softmax exps hide under the
            # batch-1 projection matmuls.
            def qkv_quarters(qs):
                with (
                    tc.tile_pool(name=f"qkv_ps{qs[0]}", bufs=6, space="PSUM") as pp,
                    tc.tile_pool(name=f"tr_ps{qs[0]}", bufs=2, space="PSUM") as trp,
                ):
                    for qi in qs:
                        for name, dest in (("v", vT_sb), ("q", qT_sb), ("k", kT_sb)):
                            for blk in range(2):
                                g = 2 * qi + blk
                                ps = pp.tile([128, 512], f32, tag="qkvps",
                                             name=f"ps_{name}{g}")
                                for k in range(NKT):
                                    nc.tensor.matmul(
                                        ps,
                                        w_sb[name][:, k, :],
                                        xc_sb[qi][k][:, 512 * blk : 512 * (blk + 1)],
                                        start=(k == 0),
                                        stop=(k == NKT - 1),
                                    )
                                nc.vector.tensor_scalar_add(
                                    dest[:, 512 * g : 512 * (g + 1)], ps,
                                    bias_sb[name],
                                )
                            if name == "v":
                                # transpose this quarter's V to [keys, dims]
                                for kb in range(8 * qi, 8 * qi + 8):
                                    tp = trp.tile([128, 128], bf16, tag="trps")
                                    nc.tensor.transpose(
                                        tp, vT_sb[:, 128 * kb : 128 * (kb + 1)],
                                        ident,
                                    )
                                    nc.vector.tensor_copy(
                                        v_ab[:, kb, :].rearrange(
                                            "p (g c) -> p g c", c=65
                                        )[:, :, 0:64],
                                        tp.rearrange("p (g c) -> p g c", c=64),
                                    )

            qkv_quarters([0, 1])

            # ---- phase 3: attention per (batch, strip-pair) ----
            # strips are processed in pairs (0,1) and (2,3); for key blocks
            # visible to both strips the scores psum is [128, 1024] (cols
            # 0-511 strip s, 512-1023 strip s+1) and exp runs once over it.
            cc_writes = {0: [], 1: [], 2: [], 3: []}
            collectives = {}
            af = {}
            afn = {}
            scl = {}
            for w in range(4):
                af[w] = singles.tile(
                    [128, NCORES * 128], bf16, tag=f"af{w}", name=f"af{w}"
                )
                afn[w] = singles.tile(
                    [128, NCORES * 128], bf16, tag=f"afn{w}", name=f"afn{w}"
                )
                scl[w] = singles.tile(
                    [128, NCORES * 128], bf16, tag=f"scl{w}", name=f"scl{w}"
                )

            def emit_collective(tag, windows, cin, cout):
                cc = nc.gpsimd.collective_compute(
                    "AllToAll",
                    mybir.AluOpType.bypass,
                    ins=[cin],
                    outs=[cout],
                    replica_groups=[list(range(NCORES))],
                )
                for w in windows:
                    for wr in cc_writes[w]:
                        add_dep_helper(cc.ins, wr.ins, sync=True,
                                       reason=f"cc{tag} in ready")
                collectives[tag] = cc

            def load_af(w, eng_af, eng_scl, after=()):
                cin, cout, row_off, sstride = wininfo[w]
                cc = collectives[win_cc_tag[w]]
                handles = []
                for hh in range(2):
                    src = _bass.AP(
                        tensor=cout.tensor,
                        offset=128 * (row_off + 65 * hh),
                        ap=[[128, 64], [128 * sstride, NCORES], [1, 128]],
                    )
                    rd = eng_af.dma_start(
                        out=af[w][64 * hh : 64 * (hh + 1), :].rearrange(
                            "p (i x) -> p i x", x=128
                        ),
                        in_=src,
                    )
                    add_dep_helper(rd.ins, cc.ins, sync=True, reason="cc out ready")
                    handles.append(rd.ins)
                    ssrc = _bass.AP(
                        tensor=cout.tensor,
                        offset=128 * (row_off + 65 * hh + 64),
                        ap=[[0, 64], [128 * sstride, NCORES], [1, 128]],
                    )
                    rd = eng_scl.dma_start(
                        out=scl[w][64 * hh : 64 * (hh + 1), :].rearrange(
                            "p (i x) -> p i x", x=128
                        ),
                        in_=ssrc,
                    )
                    add_dep_helper(rd.ins, cc.ins, sync=True, reason="cc rec ready")
                    handles.append(rd.ins)
                for hnd in handles:
                    for prior in after:
                        add_dep_helper(hnd, prior, sync=False, reason="queue order")
                return handles

            with (
                tc.tile_pool(name="p_sb", bufs=10) as ppool,
                tc.tile_pool(name="att_sb", bufs=10) as apool,
                tc.tile_pool(name="rec_sb", bufs=4) as rpool,
            ):
                last_exps = []
                af_early = []

                def emit_pair(b, s0, scp, avp):
                        s1 = s0 + 1
                        w = 2 * b + s0 // 2
                        last_pair = w == 3
                        qc1 = slice(T * b + 512 * s1, T * b + 512 * (s1 + 1))
                        psV = {}
                        for h in ("A", "B"):
                            for sx in (s0, s1):
                                psV[(h, sx)] = avp.tile(
                                    [128, 512], f32, tag="av", name=f"psV_{h}{w}{sx}"
                                )
                        nkb0, nkb1 = 4 * (s0 + 1), 4 * (s1 + 1)
                        for kb in range(nkb1):
                            krange = slice(T * b + 128 * kb, T * b + 128 * (kb + 1))
                            gkb = (T // 128) * b + kb
                            both = kb < nkb0
                            p_of = {}
                            # pass 1: scores + exp + mask for BOTH heads, so
                            # PE fills with head-B scores while head-A exps
                            for hi, h in enumerate(("A", "B")):
                                rows = slice(64 * hi, 64 * (hi + 1))
                                psS = scp.tile([128, 1024], f32, tag="sc")
                                p = ppool.tile([128, 1024], bf16, tag="p")
                                p_of[h] = p
                                scale = 1.0 / float(np.sqrt(HD))
                                if both:
                                    # cols [0:off) of the s0 half are fully
                                    # causally masked -> skip them entirely
                                    m = kb - 4 * s0
                                    off = 128 * m if m >= 0 else 0
                                    nc.tensor.matmul(
                                        psS[:, off:512],
                                        kT_sb[rows, krange],
                                        qT_sb[
                                            rows,
                                            T * b + 512 * s0 + off
                                            : T * b + 512 * (s0 + 1),
                                        ],
                                        start=True,
                                        stop=True,
                                        tile_position=(64 * hi, 0),
                                    )
                                    nc.tensor.matmul(
                                        psS[:, 512:1024],
                                        kT_sb[rows, krange],
                                        qT_sb[rows, qc1],
                                        start=True,
                                        stop=True,
                                        tile_position=(64 * hi, 0),
                                    )
                                    ei = nc.scalar.activation(
                                        out=p[:, off:1024],
                                        in_=psS[:, off:1024],
                                        func=mybir.ActivationFunctionType.Exp,
                                        scale=scale,
                                    )
                                    if last_pair:
                                        last_exps.append(ei.ins)
                                    if m >= 0:
                                        # mask strip s0 half; s1 fully visible
                                        nc.vector.tensor_mul(
                                            p[:, off:1024],
                                            p[:, off:1024],
                                            mask_sb[
                                                :,
                                                1024 * m + off : 1024 * (m + 1),
                                            ],
                                        )
                                else:
                                    m = kb - 4 * s1
                                    off = 128 * m if m >= 0 else 0
                                    nc.tensor.matmul(
                                        psS[:, 512 + off : 1024],
                                        kT_sb[rows, krange],
                                        qT_sb[
                                            rows,
                                            T * b + 512 * s1 + off
                                            : T * b + 512 * (s1 + 1),
                                        ],
                                        start=True,
                                        stop=True,
                                        tile_position=(64 * hi, 0),
                                    )
                                    ei = nc.scalar.activation(
                                        out=p[:, 512 + off : 1024],
                                        in_=psS[:, 512 + off : 1024],
                                        func=mybir.ActivationFunctionType.Exp,
                                        scale=scale,
                                    )
                                    if last_pair:
                                        last_exps.append(ei.ins)
                                    if m >= 0:
                                        nc.vector.tensor_mul(
                                            p[:, 512 + off : 1024],
                                            p[:, 512 + off : 1024],
                                            mask_sb[
                                                :,
                                                1024 * m + off : 1024 * m + 512,
                                            ],
                                        )
                            # pass 2: attnV accumulate; lhsT = [v_h | ones]:
                            # attnout rows 0-63, softmax sums row 64.
                            m = kb - 4 * (s0 if both else s1)
                            off = 128 * m if m >= 0 else 0
                            for hi, h in enumerate(("A", "B")):
                                p = p_of[h]
                                lhsT = v_ab[:, gkb, 65 * hi : 65 * hi + 65]
                                if both:
                                    nc.tensor.matmul(
                                        psV[(h, s0)][0:65, off:512],
                                        lhsT,
                                        p[:, off:512],
                                        start=(kb == 0),
                                        stop=(kb == nkb0 - 1),
                                    )
                                    nc.tensor.matmul(
                                        psV[(h, s1)][0:65, 0:512],
                                        lhsT,
                                        p[:, 512:1024],
                                        start=(kb == 0),
                                        stop=(kb == nkb1 - 1),
                                    )
                                else:
                                    nc.tensor.matmul(
                                        psV[(h, s1)][0:65, off:512],
                                        lhsT,
                                        p[:, 512 + off : 1024],
                                        start=False,
                                        stop=(kb == nkb1 - 1),
                                    )
                        # ship unnormalized attn-out + reciprocal rows into
                        # this window's collective buffer.
                        cin, cout, row_off, sstride = wininfo[w]
                        for sx in (s0, s1):
                            base_j = 4 * (sx % 2)
                            if sx % 2 == 0:
                                eng = nc.sync
                            elif last_pair:
                                # ACT queue is free once the pair's exps are
                                # done; avoids serial Pool SWDGE desc-gen on
                                # the critical chain into the last collective
                                eng = nc.scalar
                            else:
                                eng = nc.gpsimd
                            # per head: [attn-out rows 0:64 | reciprocal row
                            # 64], shipped as ONE dma into the shard's 65-row
                            # head group.  Copies split across DVE/Pool on the
                            # last pair so the final drain chain is shorter.
                            for hi, h in enumerate(("A", "B")):
                                att = apool.tile([65, 512], bf16, tag="att")
                                nc.vector.tensor_copy(att[0:64, :], psV[(h, sx)][0:64, :])
                                with nc.allow_low_precision("bf16 softmax recip"):
                                    nc.vector.reciprocal(
                                        att[64:65, :], psV[(h, sx)][64:65, :]
                                    )
                                dst = _bass.AP(
                                    tensor=cin.tensor,
                                    offset=128 * (sstride * base_j + row_off + 65 * hi),
                                    ap=[[128, 65], [128 * sstride, 4], [1, 128]],
                                )
                                wr = eng.dma_start(
                                    out=dst,
                                    in_=att.rearrange("p (c x) -> p c x", x=128),
                                )
                                if last_pair and sx % 2 == 1:
                                    for e in last_exps:
                                        add_dep_helper(wr.ins, e, sync=False,
                                                       reason="act q order")
                                cc_writes[w].append(wr)
                        # issue collectives as their windows complete
                        if w == 0:
                            emit_collective("A", [0], ccA_in, ccA_out)
                            af_early.extend(
                                load_af(
                                    0, nc.sync, nc.sync,
                                    after=[wr.ins for wr in cc_writes[0]],
                                )
                            )
                        elif w == 2:
                            emit_collective("B", [1, 2], ccB_in, ccB_out)
                        elif w == 3:
                            emit_collective("C", [3], ccC_in, ccC_out)

                qkv_quarters([2, 3])
                with (
                    tc.tile_pool(name="sc_ps1", bufs=2, space="PSUM") as scp1,
                    tc.tile_pool(name="av_ps1", bufs=4, space="PSUM") as avp1,
                ):
                    emit_pair(0, 0, scp1, avp1)
                    emit_pair(0, 2, scp1, avp1)
                    emit_pair(1, 0, scp1, avp1)
                    emit_pair(1, 2, scp1, avp1)

            # ---- phase 5: output projection, one 128-row block per window.
            with (
                tc.tile_pool(name="op_ps", bufs=2, space="PSUM") as op,
                tc.tile_pool(name="out_sb", bufs=2) as opool,
            ):
                warm = op.tile([128, 512], f32, tag="op", name="warm_ps")
                out_dmas = []
                copies = []

                def compute_outproj(w):
                    nc.vector.tensor_mul(afn[w], af[w], scl[w])
                    o_sb = opool.tile([128, D], f32, tag="osb")
                    for n in range(D // 512):
                        ps = op.tile([128, 512], f32, tag="op")
                        for i in range(NCORES):
                            nc.tensor.matmul(
                                ps,
                                afn[w][:, 128 * i : 128 * (i + 1)],
                                wo_sb[:, i, 512 * n : 512 * (n + 1)],
                                start=(i == 0),
                                stop=False,
                            )
                        nc.tensor.matmul(
                            ps,
                            ones_row,
                            bo_sb[:, 512 * n : 512 * (n + 1)],
                            start=False,
                            stop=True,
                        )
                        cp = nc.scalar.copy(o_sb[:, 512 * n : 512 * (n + 1)], ps)
                        copies.append(cp.ins)
                    od = [nc.sync, nc.gpsimd][w % 2 if w != 3 else 0].dma_start(
                        out=out[128 * w : 128 * (w + 1), :], in_=o_sb
                    )
                    out_dmas.append(od.ins)

                def pe_warm(n):
                    # keep the PE array busy/ramped while a collective drains
                    for _ in range(n):
                        nc.tensor.matmul(
                            warm, ident, kT_sb[:, 0:512], start=True, stop=True
                        )

                # windows 0-2 must not be hoisted ahead of the tail attention
                # writes they share queues with
                tail_writes = [wr.ins for wr in cc_writes[3]]
                # keep the early af0 load (emitted mid-attention) ahead of the
                # later attention DMAs on SP so outproj(0) is ready at
                # attention end
                for wr in cc_writes[2] + cc_writes[3]:
                    for hnd in af_early:
                        add_dep_helper(wr.ins, hnd, sync=False,
                                       reason="af0 before tail writes")
                compute_outproj(0)
                load_af(1, nc.sync, nc.sync, after=tail_writes)
                load_af(2, nc.sync, nc.sync, after=tail_writes)
                pe_warm(12)
                compute_outproj(1)
                compute_outproj(2)
                load_af(3, nc.sync, nc.scalar,
                        after=tail_writes + out_dmas + copies + last_exps)
                pe_warm(89)
                compute_outproj(3)

    return nc


def _host_prep(x, Wq, bq, Wk, bk, Wv, bv, Wo, bo):
    """Build the 8 per-core input maps."""
    x = np.asarray(x, np.float32)
    xT = np.ascontiguousarray(x.reshape(R, D).T).astype(_BF16)
    woT = np.ascontiguousarray(np.asarray(Wo, np.float32).T).astype(_BF16)
    bo_row = np.asarray(bo, np.float32).reshape(1, D).astype(_BF16)

    in_maps = []
    for core in range(NCORES):
        hs = slice(HDIM * core, HDIM * (core + 1))
        in_maps.append(
            {
                "xT": xT,
                "wqT": np.ascontiguousarray(np.asarray(Wq, np.float32)[hs, :].T).astype(_BF16),
                "wkT": np.ascontiguousarray(np.asarray(Wk, np.float32)[hs, :].T).astype(_BF16),
                "wvT": np.ascontiguousarray(np.asarray(Wv, np.float32)[hs, :].T).astype(_BF16),
                "bq_s": np.asarray(bq, np.float32)[hs].reshape(HDIM, 1).copy(),
                "bk_s": np.asarray(bk, np.float32)[hs].reshape(HDIM, 1).copy(),
                "bv_s": np.asarray(bv, np.float32)[hs].reshape(HDIM, 1).copy(),
                "woT": woT,
                "bo_row": bo_row,
            }
        )
    return in_maps


def _run(in_maps, trace=False):
    from concourse import bass_utils

    if "nc" not in _cache:
        _cache["nc"] = _build()
    nc = _cache["nc"]
    if trace:
        try:
            res = bass_utils.run_bass_kernel_spmd(
                nc, in_maps, core_ids=list(range(NCORES)), trace=True
            )
            return res
        except Exception:
            pass  # NTFF hook unavailable under this axon build
    try:
        res = bass_utils.run_bass_kernel_spmd(
            nc, in_maps, core_ids=list(range(NCORES)), trace=False
        )
    except Exception:
        # transient device faults (NRT_EXEC_UNIT_UNRECOVERABLE) clear on retry
        res = bass_utils.run_bass_kernel_spmd(
            nc, in_maps, core_ids=list(range(NCORES)), trace=False
        )
    return res


def kernel(x, Wq, bq, Wk, bk, Wv, bv, Wo, bo, _trace=False, _want_results=False):
    in_maps = _host_prep(x, Wq, bq, Wk, bk, Wv, bv, Wo, bo)
    res = _run(in_maps, trace=_trace)
    # core j's out rows 128w..128w+128 are global rows 1024w + 128j ..+128
    parts = np.stack(
        [np.asarray(res.results[c]["out"]).reshape(4, 128, D) for c in range(NCORES)]
    )  # [j, w, r, D]
    full = (
        parts.transpose(1, 0, 2, 3).reshape(B, T, D).astype(np.float32)
    )
    if _want_results:
        return full, res
    return full


# revision 46
# speedup vs baseline: 1.0020x; 1.0020x over previous
"""Causal self-attention on 8 trn2 NeuronCores.

Sharding: tensor-parallel over heads (2 heads/core) for QKV+attention.  Row
ownership for the output projection is window-interleaved: window w = global
rows [1024w, 1024w+1024) and core j owns rows 1024w+128j..+128 of every
window.  This makes the head-split -> row-split reshard expressible as FOUR
per-window 8-rank AllToAlls that are issued as soon as each strip-pair's
attention completes, hiding most of the collective cost under attention
compute (windows 1+2 are merged into one collective; the last window's
collective is small and constant-overhead-dominated).

Softmax normalization moves to the RECEIVE side of the collective: each core
ships unnormalized attn-out rows plus per-(head,row) reciprocal rows, and the
receiver scales the gathered [1024 dims x 128 rows] tile once before the
output projection.

All matmuls run in bf16 with f32 PSUM accumulation.  Attention is computed in
"scores transposed" layout ([keys, queries] on chip); softmax denominators
come from a ones column appended to V, and the causal mask is a
multiplicative {0,1} bf16 mask (generated on-chip via affine_select) applied
after exp (safe: scores are O(6), no overflow without max-subtraction).
"""

import numpy as np
import ml_dtypes

B, T, D, H, HD = 2, 2048, 1024, 16, 64
NCORES = 8
R = B * T              # 4096 global rows (b*T + t)
HPC = H // NCORES      # 2 heads per core
HDIM = HPC * HD        # 128 dims per core
ROWS_PER_CORE = R // NCORES  # 512
NKT = D // 128         # 8 contraction tiles
NSTRIP = T // 512      # 4 query strips per batch
WPS = 130              # rows per (window, shard) in cc buffers: 128 dims + 2 rec

_BF16 = ml_dtypes.bfloat16
_cache = {}


def _patch_tile_drain():
    """This walrus build rejects >1 sync wait on SP CTRL instructions; split
    the Tile tail-drain waits across single-wait nops."""
    import concourse.mybir as mybir
    import concourse.tile as tile_mod
    from concourse.vector_clock import ScopedClock

    if getattr(tile_mod.TileContext, "_drain_patched", False):
        return

    def _drain_and_barrier(self, tick_clock, wait_clock):
        nc = self.nc
        dummy = mybir.InstNoOp(
            name=nc.get_next_instruction_name(),
            engine=mybir.EngineType.SP,
            ins=[],
            outs=[],
        )
        wait_clock.add_sem_waits(dummy, ScopedClock({None: tick_clock.global_clock}))
        waits = list(dummy.sync_info.on_wait) if dummy.sync_info else []
        for i in range(len(waits)):
            w = nc.sync.nop(nofuse=True, hint="tail_drain_wait")
            w.ins.sync_info = mybir.SyncInfo(on_wait=waits[i : i + 1], on_update=[])
        nc.sync.drain()
        nc.all_engine_barrier()
        assert self.sems is not None
        popped = nc._tile_sem_poison_stack.pop()
        assert popped is self._sem_poison
        nc.clear_and_free_semaphores(list(self.sems.allocated().values()))
        nc.all_engine_barrier()

    tile_mod.TileContext._drain_and_barrier = _drain_and_barrier

    # Body instructions can also accumulate >2 waits (CTRL structs take 1,
    # other structs 2 on this walrus).  Before lowering, move excess waits
    # onto single-wait nops inserted just before the instruction on the same
    # engine stream.
    _orig_lower = tile_mod.TileContext._lower_ordered_insts

    def _lower_split_waits(self, ordered):
        nc = self.nc
        for bb_name, insts in ordered.items():
            new_insts = []
            for inst in insts:
                si = getattr(inst, "sync_info", None)
                waits = list(si.on_wait) if si is not None and si.on_wait else []
                limit = 1
                if len(waits) > limit and inst.engine is not None:
                    keep = waits[: limit - 1] if limit > 1 else []
                    spill = waits[len(keep) :][:-1]
                    keep = keep + [waits[-1]]
                    for w in spill:
                        nop = mybir.InstNoOp(
                            name=nc.get_next_instruction_name(),
                            engine=inst.engine,
                            ins=[],
                            outs=[],
                        )
                        nop.sync_info = mybir.SyncInfo(on_wait=[w], on_update=[])
                        nop.debug = inst.debug
                        new_insts.append(nop)
                    inst.sync_info = mybir.SyncInfo(
                        on_wait=keep, on_update=list(si.on_update or [])
                    )
                new_insts.append(inst)
            ordered[bb_name] = new_insts
        return _orig_lower(self, ordered)

    tile_mod.TileContext._lower_ordered_insts = _lower_split_waits
    tile_mod.TileContext._drain_patched = True


def _build():
    import concourse.bass as bass
    import concourse.mybir as mybir
    import concourse.tile as tile
    from concourse.tile import add_dep_helper
    from concourse.masks import make_identity

    _patch_tile_drain()
    f32 = mybir.dt.float32
    bf16 = mybir.dt.bfloat16

    nc = bass.Bass("TRN2", target_bir_lowering=False, debug=False, num_devices=NCORES)

    # ---- DRAM I/O (per core) ----
    xT = nc.dram_tensor("xT", [D, R], bf16, kind="ExternalInput").ap()
    wqT = nc.dram_tensor("wqT", [D, HDIM], bf16, kind="ExternalInput").ap()
    wkT = nc.dram_tensor("wkT", [D, HDIM], bf16, kind="ExternalInput").ap()
    wvT = nc.dram_tensor("wvT", [D, HDIM], bf16, kind="ExternalInput").ap()
    bq_s = nc.dram_tensor("bq_s", [HDIM, 1], f32, kind="ExternalInput").ap()
    bk_s = nc.dram_tensor("bk_s", [HDIM, 1], f32, kind="ExternalInput").ap()
    bv_s = nc.dram_tensor("bv_s", [HDIM, 1], f32, kind="ExternalInput").ap()
    woT = nc.dram_tensor("woT", [D, D], bf16, kind="ExternalInput").ap()
    bo_row = nc.dram_tensor("bo_row", [1, D], bf16, kind="ExternalInput").ap()
    out = nc.dram_tensor("out", [ROWS_PER_CORE, D], f32, kind="ExternalOutput").ap()

    # collective bounce buffers. windows: w = 2b + s0//2 covers global rows
    # [1024w, 1024w+1024); shard j = core j's 128 owned rows of the window.
    # shard layout (WPS=130 rows): 0:128 = 2-head dims, 128:130 = reciprocals.
    # groups: A=[w0], B=[w1,w2] (shard = w1 130 rows then w2 130 rows), C=[w3].
    ccA_in = nc.dram_tensor("ccA_in", [NCORES * WPS, 128], bf16).ap()
    ccA_out = nc.dram_tensor("ccA_out", [NCORES * WPS, 128], bf16).ap()
    ccB_in = nc.dram_tensor("ccB_in", [NCORES * 2 * WPS, 128], bf16).ap()
    ccB_out = nc.dram_tensor("ccB_out", [NCORES * 2 * WPS, 128], bf16).ap()
    ccC_in = nc.dram_tensor("ccC_in", [NCORES * WPS, 128], bf16).ap()
    ccC_out = nc.dram_tensor("ccC_out", [NCORES * WPS, 128], bf16).ap()
    # per-window: (in_ap, out_ap, row offset inside shard, shard stride rows)
    wininfo = {
        0: (ccA_in, ccA_out, 0, WPS),
        1: (ccB_in, ccB_out, 0, 2 * WPS),
        2: (ccB_in, ccB_out, WPS, 2 * WPS),
        3: (ccC_in, ccC_out, 0, WPS),
    }
    win_cc_tag = {0: "A", 1: "B", 2: "B", 3: "C"}

    with tile.TileContext(nc) as tc:
        import contextlib
        import concourse.bass as _bass

        with contextlib.ExitStack() as ctx:
            singles = ctx.enter_context(tc.tile_pool(name="singles", bufs=1))

            # ---- weights on fast queues; x chunks fill the two HWDGE queues
            # (the gpsimd SWDGE queue pays ~1us of Pool desc-gen per DMA, so
            # it only carries the non-urgent loads: wq/biases/bo/wo).
            w_sb = {}
            for (name, src), eng in zip(
                (("v", wvT), ("q", wqT), ("k", wkT)),
                (nc.sync, nc.gpsimd, nc.scalar),
            ):
                t = singles.tile([128, NKT, HDIM], bf16, tag=f"w{name}", name=f"w{name}")
                eng.dma_start(out=t, in_=src.rearrange("(k p) c -> p k c", p=128))
                w_sb[name] = t
            bias_sb = {}
            for name, src in (("q", bq_s), ("k", bk_s), ("v", bv_s)):
                t = singles.tile([HDIM, 1], f32, tag=f"b{name}", name=f"b{name}")
                nc.gpsimd.dma_start(out=t, in_=src)
                bias_sb[name] = t
            # x in 32 chunks [128, 1024], quarter-major k-minor so quarter 0's
            # contraction tiles land first and QKV starts ~5us in.
            feed_engs = [nc.sync, nc.gpsimd, nc.scalar]
            xc_sb = [[None] * NKT for _ in range(4)]
            for qi in range(4):
                for k in range(NKT):
                    t = singles.tile([128, 1024], bf16, tag=f"xc{qi}_{k}")
                    if qi == 0:
                        # quarter 0 gates the first matmuls: keep it off the
                        # slow gpsimd SWDGE queue
                        eng = [nc.sync, nc.scalar][k % 2]
                    else:
                        eng = feed_engs[(NKT * qi + k) % 3]
                    eng.dma_start(
                        out=t,
                        in_=xT[128 * k : 128 * (k + 1), 1024 * qi : 1024 * (qi + 1)],
                    )
                    xc_sb[qi][k] = t
            bo_sb = singles.tile([1, D], bf16, tag="bo")
            nc.gpsimd.dma_start(out=bo_sb, in_=bo_row)
            wo_sb = singles.tile([128, NKT, D], bf16, tag="wo")
            nc.sync.dma_start(out=wo_sb, in_=woT.rearrange("(k p) c -> p k c", p=128))

            # ---- on-chip constants ----
            ident = singles.tile([128, 128], bf16, tag="ident")
            make_identity(nc, ident)
            ones_row = singles.tile([1, 128], bf16, tag="ones")
            nc.vector.memset(ones_row, 1.0)
            # causal masks: block m is [mask_m(512) | ones(512)];
            # mask_m[r, c] = 1.0 iff c - r - 128m >= 0.
            mask_sb = singles.tile([128, 4 * 1024], bf16, tag="mask")
            nc.gpsimd.memset(mask_sb, 1.0)
            mones = singles.tile([128, 512], bf16, tag="mones")
            nc.gpsimd.memset(mones, 1.0)
            for m in range(4):
                nc.gpsimd.affine_select(
                    out=mask_sb[:, 1024 * m : 1024 * m + 512],
                    in_=mones,
                    pattern=[[1, 512]],
                    compare_op=mybir.AluOpType.is_ge,
                    fill=0.0,
                    base=-128 * m,
                    channel_multiplier=-1,
                )
            # preload the Exp activation table off the critical path
            exp_warm = singles.tile([1, 128], bf16, tag="expwarm")
            nc.scalar.activation(
                out=exp_warm,
                in_=ones_row,
                func=mybir.ActivationFunctionType.Exp,
                scale=1.0,
            )

            # persistent activations
            qT_sb = singles.tile([128, R], bf16, tag="qT")   # rows 0-63 head A dims
            kT_sb = singles.tile([128, R], bf16, tag="kT")
            vT_sb = singles.tile([128, R], bf16, tag="vT")
            # v in [keys, dims] layout per key block kb:
            #   cols 0:64 = vA, 64 = ones, 65:129 = vB, 129 = ones
            # so lhsT for head h = cols [65h, 65h+65) = [v_h | ones]:
            # attnout at psum rows 0-63, softmax sums at row 64.
            v_ab = singles.tile([128, R // 128, 130], bf16, tag="vab")
            nc.vector.memset(v_ab[:, :, 64:65], 1.0)
            nc.vector.memset(v_ab[:, :, 129:130], 1.0)

            # ---- phase 1: projections, quarter-by-quarter so compute starts
            # as soon as the first 1MiB of x lands.  V first per quarter so
            # its transposes stay off the critical path.  Emitted in two
            # batch-halves so pair-1 attention (which only needs batch 0)
            # interleaves between them and its softmax exps hide under the
            # batch-1 projection matmuls.
            def qkv_quarters(qs):
                with (
                    tc.tile_pool(name=f"qkv_ps{qs[0]}", bufs=6, space="PSUM") as pp,
                    tc.tile_pool(name=f"tr_ps{qs[0]}", bufs=2, space="PSUM") as trp,
                ):
                    for qi in qs:
                        for name, dest in (("v", vT_sb), ("q", qT_sb), ("k", kT_sb)):
                            for blk in range(2):
                                g = 2 * qi + blk
                                ps = pp.tile([128, 512], f32, tag="qkvps",
                                             name=f"ps_{name}{g}")
                                for k in range(NKT):
                                    nc.tensor.matmul(
                                        ps,
                                        w_sb[name][:, k, :],
                                        xc_sb[qi][k][:, 512 * blk : 512 * (blk + 1)],
                                        start=(k == 0),
                                        stop=(k == NKT - 1),
                                    )
                                nc.vector.tensor_scalar_add(
                                    dest[:, 512 * g : 512 * (g + 1)], ps,
                                    bias_sb[name],
                                )
                            if name == "v":
                                # transpose this quarter's V to [keys, dims]
                                for kb in range(8 * qi, 8 * qi + 8):
                                    tp = trp.tile([128, 128], bf16, tag="trps")
                                    nc.tensor.transpose(
                                        tp, vT_sb[:, 128 * kb : 128 * (kb + 1)],
                                        ident,
                                    )
                                    nc.vector.tensor_copy(
                                        v_ab[:, kb, :].rearrange(
                                            "p (g c) -> p g c", c=65
                                        )[:, :, 0:64],
                                        tp.rearrange("p (g c) -> p g c", c=64),
                                    )

            qkv_quarters([0, 1])

            # ---- phase 3: attention per (batch, strip-pair) ----
            # strips are processed in pairs (0,1) and (2,3); for key blocks
            # visible to both strips the scores psum is [128, 1024] (cols
            # 0-511 strip s, 512-1023 strip s+1) and exp runs once over it.
            cc_writes = {0: [], 1: [], 2: [], 3: []}
            collectives = {}
            af = {}
            afn = {}
            scl = {}
            for w in range(4):
                af[w] = singles.tile(
                    [128, NCORES * 128], bf16, tag=f"af{w}", name=f"af{w}"
                )
                afn[w] = singles.tile(
                    [128, NCORES * 128], bf16, tag=f"afn{w}", name=f"afn{w}"
                )
                scl[w] = singles.tile(
                    [128, NCORES * 128], bf16, tag=f"scl{w}", name=f"scl{w}"
                )

            def emit_collective(tag, windows, cin, cout):
                cc = nc.gpsimd.collective_compute(
                    "AllToAll",
                    mybir.AluOpType.bypass,
                    ins=[cin],
                    outs=[cout],
                    replica_groups=[list(range(NCORES))],
                )
                for w in windows:
                    for wr in cc_writes[w]:
                        add_dep_helper(cc.ins, wr.ins, sync=True,
                                       reason=f"cc{tag} in ready")
                collectives[tag] = cc

            def load_af(w, eng_af, eng_scl, after=()):
                cin, cout, row_off, sstride = wininfo[w]
                cc = collectives[win_cc_tag[w]]
                handles = []
                for hh in range(2):
                    src = _bass.AP(
                        tensor=cout.tensor,
                        offset=128 * (row_off + 65 * hh),
                        ap=[[128, 64], [128 * sstride, NCORES], [1, 128]],
                    )
                    rd = eng_af.dma_start(
                        out=af[w][64 * hh : 64 * (hh + 1), :].rearrange(
                            "p (i x) -> p i x", x=128
                        ),
                        in_=src,
                    )
                    add_dep_helper(rd.ins, cc.ins, sync=True, reason="cc out ready")
                    handles.append(rd.ins)
                    ssrc = _bass.AP(
                        tensor=cout.tensor,
                        offset=128 * (row_off + 65 * hh + 64),
                        ap=[[0, 64], [128 * sstride, NCORES], [1, 128]],
                    )
                    rd = eng_scl.dma_start(
                        out=scl[w][64 * hh : 64 * (hh + 1), :].rearrange(
                            "p (i x) -> p i x", x=128
                        ),
                        in_=ssrc,
                    )
                    add_dep_helper(rd.ins, cc.ins, sync=True, reason="cc rec ready")
                    handles.append(rd.ins)
                for hnd in handles:
                    for prior in after:
                        add_dep_helper(hnd, prior, sync=False, reason="queue order")
                return handles

            with (
                tc.tile_pool(name="p_sb", bufs=10) as ppool,
                tc.tile_pool(name="att_sb", bufs=10) as apool,
                tc.tile_pool(name="rec_sb", bufs=4) as rpool,
            ):
                last_exps = []
                af_early = []

                def emit_pair(b, s0, scp, avp):
                        s1 = s0 + 1
                        w = 2 * b + s0 // 2
                        last_pair = w == 3
                        qc1 = slice(T * b + 512 * s1, T * b + 512 * (s1 + 1))
                        psV = {}
                        for h in ("A", "B"):
                            for sx in (s0, s1):
                                psV[(h, sx)] = avp.tile(
                                    [128, 512], f32, tag="av", name=f"psV_{h}{w}{sx}"
                                )
                        nkb0, nkb1 = 4 * (s0 + 1), 4 * (s1 + 1)
                        for kb in range(nkb1):
                            krange = slice(T * b + 128 * kb, T * b + 128 * (kb + 1))
                            gkb = (T // 128) * b + kb
                            both = kb < nkb0
                            p_of = {}
                            # pass 1: scores + exp + mask for BOTH heads, so
                            # PE fills with head-B scores while head-A exps
                            for hi, h in enumerate(("A", "B")):
                                rows = slice(64 * hi, 64 * (hi + 1))
                                psS = scp.tile([128, 1024], f32, tag="sc")
                                p = ppool.tile([128, 1024], bf16, tag="p")
                                p_of[h] = p
                                scale = 1.0 / float(np.sqrt(HD))
                                if both:
                                    # cols [0:off) of the s0 half are fully
                                    # causally masked -> skip them entirely
                                    m = kb - 4 * s0
                                    off = 128 * m if m >= 0 else 0
                                    nc.tensor.matmul(
                                        psS[:, off:512],
                                        kT_sb[rows, krange],
                                        qT_sb[
                                            rows,
                                            T * b + 512 * s0 + off
                                            : T * b + 512 * (s0 + 1),
                                        ],
                                        start=True,
                                        stop=True,
                                        tile_position=(64 * hi, 0),
                                    )
                                    nc.tensor.matmul(
                                        psS[:, 512:1024],
                                        kT_sb[rows, krange],
                                        qT_sb[rows, qc1],
                                        start=True,
                                        stop=True,
                                        tile_position=(64 * hi, 0),
                                    )
                                    ei = nc.scalar.activation(
                                        out=p[:, off:1024],
                                        in_=psS[:, off:1024],
                                        func=mybir.ActivationFunctionType.Exp,
                                        scale=scale,
                                    )
                                    if last_pair:
                                        last_exps.append(ei.ins)
                                    if m >= 0:
                                        # mask strip s0 half; s1 fully visible
                                        nc.vector.tensor_mul(
                                            p[:, off:1024],
                                            p[:, off:1024],
                                            mask_sb[
                                                :,
                                                1024 * m + off : 1024 * (m + 1),
                                            ],
                                        )
                                else:
                                    m = kb - 4 * s1
                                    off = 128 * m if m >= 0 else 0
                                    nc.tensor.matmul(
                                        psS[:, 512 + off : 1024],
                                        kT_sb[rows, krange],
                                        qT_sb[
                                            rows,
                                            T * b + 512 * s1 + off
                                            : T * b + 512 * (s1 + 1),
                                        ],
                                        start=True,
                                        stop=True,
                                        tile_position=(64 * hi, 0),
                                    )
                                    ei = nc.scalar.activation(
                                        out=p[:, 512 + off : 1024],
                                        in_=psS[:, 512 + off : 1024],
                                        func=mybir.ActivationFunctionType.Exp,
                                        scale=scale,
                                    )
                                    if last_pair:
                                        last_exps.append(ei.ins)
                                    if m >= 0:
                                        nc.vector.tensor_mul(
                                            p[:, 512 + off : 1024],
                                            p[:, 512 + off : 1024],
                                            mask_sb[
                                                :,
                                                1024 * m + off : 1024 * m + 512,
                                            ],
                                        )
                            # pass 2: attnV accumulate; lhsT = [v_h | ones]:
                            # attnout rows 0-63, softmax sums row 64.
                            m = kb - 4 * (s0 if both else s1)
                            off = 128 * m if m >= 0 else 0
                            for hi, h in enumerate(("A", "B")):
                                p = p_of[h]
                                lhsT = v_ab[:, gkb, 65 * hi : 65 * hi + 65]
                                if both:
                                    nc.tensor.matmul(
                                        psV[(h, s0)][0:65, off:512],
                                        lhsT,
                                        p[:, off:512],
                                        start=(kb == 0),
                                        stop=(kb == nkb0 - 1),
                                    )
                                    nc.tensor.matmul(
                                        psV[(h, s1)][0:65, 0:512],
                                        lhsT,
                                        p[:, 512:1024],
                                        start=(kb == 0),
                                        stop=(kb == nkb1 - 1),
                                    )
                                else:
                                    nc.tensor.matmul(
                                        psV[(h, s1)][0:65, off:512],
                                        lhsT,
                                        p[:, 512 + off : 1024],
                                        start=False,
                                        stop=(kb == nkb1 - 1),
                                    )
                        # ship unnormalized attn-out + reciprocal rows into
                        # this window's collective buffer.
                        cin, cout, row_off, sstride = wininfo[w]
                        for sx in (s0, s1):
                            base_j = 4 * (sx % 2)
                            if sx % 2 == 0:
                                eng = nc.sync
                            elif last_pair:
                                # ACT queue is free once the pair's exps are
                                # done; avoids serial Pool SWDGE desc-gen on
                                # the critical chain into the last collective
                                eng = nc.scalar
                            else:
                                eng = nc.gpsimd
                            # per head: [attn-out rows 0:64 | reciprocal row
                            # 64], shipped as ONE dma into the shard's 65-row
                            # head group.  Copies split across DVE/Pool on the
                            # last pair so the final drain chain is shorter.
                            for hi, h in enumerate(("A", "B")):
                                att = apool.tile([65, 512], bf16, tag="att")
                                nc.vector.tensor_copy(att[0:64, :], psV[(h, sx)][0:64, :])
                                with nc.allow_low_precision("bf16 softmax recip"):
                                    nc.vector.reciprocal(
                                        att[64:65, :], psV[(h, sx)][64:65, :]
                                    )
                                dst = _bass.AP(
                                    tensor=cin.tensor,
                                    offset=128 * (sstride * base_j + row_off + 65 * hi),
                                    ap=[[128, 65], [128 * sstride, 4], [1, 128]],
                                )
                                wr = eng.dma_start(
                                    out=dst,
                                    in_=att.rearrange("p (c x) -> p c x", x=128),
                                )
                                if last_pair and sx % 2 == 1:
                                    for e in last_exps:
                                        add_dep_helper(wr.ins, e, sync=False,
                                                       reason="act q order")
                                cc_writes[w].append(wr)
                        # issue collectives as their windows complete
                        if w == 0:
                            emit_collective("A", [0], ccA_in, ccA_out)
                            af_early.extend(
                                load_af(
                                    0, nc.sync, nc.sync,
                                    after=[wr.ins for wr in cc_writes[0]],
                                )
                            )
                        elif w == 2:
                            emit_collective("B", [1, 2], ccB_in, ccB_out)
                        elif w == 3:
                            emit_collective("C", [3], ccC_in, ccC_out)

                qkv_quarters([2, 3])
                with (
                    tc.tile_pool(name="sc_ps1", bufs=2, space="PSUM") as scp1,
                    tc.tile_pool(name="av_ps1", bufs=4, space="PSUM") as avp1,
                ):
                    emit_pair(0, 0, scp1, avp1)
                    emit_pair(0, 2, scp1, avp1)
                    emit_pair(1, 0, scp1, avp1)
                    emit_pair(1, 2, scp1, avp1)

            # ---- phase 5: output projection, one 128-row block per window.
            with (
                tc.tile_pool(name="op_ps", bufs=2, space="PSUM") as op,
                tc.tile_pool(name="out_sb", bufs=2) as opool,
            ):
                warm = op.tile([128, 512], f32, tag="op", name="warm_ps")
                out_dmas = []
                copies = []

                def compute_outproj(w):
                    nc.vector.tensor_mul(afn[w], af[w], scl[w])
                    o_sb = opool.tile([128, D], f32, tag="osb")
                    for n in range(D // 512):
                        ps = op.tile([128, 512], f32, tag="op")
                        for i in range(NCORES):
                            nc.tensor.matmul(
                                ps,
                                afn[w][:, 128 * i : 128 * (i + 1)],
                                wo_sb[:, i, 512 * n : 512 * (n + 1)],
                                start=(i == 0),
                                stop=False,
                            )
                        nc.tensor.matmul(
                            ps,
                            ones_row,
                            bo_sb[:, 512 * n : 512 * (n + 1)],
                            start=False,
                            stop=True,
                        )
                        cp = nc.scalar.copy(o_sb[:, 512 * n : 512 * (n + 1)], ps)
                        copies.append(cp.ins)
                        if w == 3:
                            # the last window's out DMA is the critical tail:
                            # ship each half as soon as its copy lands
                            od = nc.sync.dma_start(
                                out=out[
                                    128 * w : 128 * (w + 1),
                                    512 * n : 512 * (n + 1),
                                ],
                                in_=o_sb[:, 512 * n : 512 * (n + 1)],
                            )
                            out_dmas.append(od.ins)
                    if w != 3:
                        od = [nc.sync, nc.gpsimd][w % 2].dma_start(
                            out=out[128 * w : 128 * (w + 1), :], in_=o_sb
                        )
                        out_dmas.append(od.ins)

                def pe_warm(n):
                    # keep the PE array busy/ramped while a collective drains
                    for _ in range(n):
                        nc.tensor.matmul(
                            warm, ident, kT_sb[:, 0:512], start=True, stop=True
                        )

                # windows 0-2 must not be hoisted ahead of the tail attention
                # writes they share queues with
                tail_writes = [wr.ins for wr in cc_writes[3]]
                # keep the early af0 load (emitted mid-attention) ahead of the
                # later attention DMAs on SP so outproj(0) is ready at
                # attention end
                for wr in cc_writes[2] + cc_writes[3]:
                    for hnd in af_early:
                        add_dep_helper(wr.ins, hnd, sync=False,
                                       reason="af0 before tail writes")
                compute_outproj(0)
                load_af(1, nc.sync, nc.sync, after=tail_writes)
                load_af(2, nc.sync, nc.sync, after=tail_writes)
                pe_warm(12)
                compute_outproj(1)
                compute_outproj(2)
                load_af(3, nc.sync, nc.scalar,
                        after=tail_writes + out_dmas + copies + last_exps)
                pe_warm(89)
                compute_outproj(3)

    return nc


def _host_prep(x, Wq, bq, Wk, bk, Wv, bv, Wo, bo):
    """Build the 8 per-core input maps."""
    x = np.asarray(x, np.float32)
    xT = np.ascontiguousarray(x.reshape(R, D).T).astype(_BF16)
    woT = np.ascontiguousarray(np.asarray(Wo, np.float32).T).astype(_BF16)
    bo_row = np.asarray(bo, np.float32).reshape(1, D).astype(_BF16)

    in_maps = []
    for core in range(NCORES):
        hs = slice(HDIM * core, HDIM * (core + 1))
        in_maps.append(
            {
                "xT": xT,
                "wqT": np.ascontiguousarray(np.asarray(Wq, np.float32)[hs, :].T).astype(_BF16),
                "wkT": np.ascontiguousarray(np.asarray(Wk, np.float32)[hs, :].T).astype(_BF16),
                "wvT": np.ascontiguousarray(np.asarray(Wv, np.float32)[hs, :].T).astype(_BF16),
                "bq_s": np.asarray(bq, np.float32)[hs].reshape(HDIM, 1).copy(),
                "bk_s": np.asarray(bk, np.float32)[hs].reshape(HDIM, 1).copy(),
                "bv_s": np.asarray(bv, np.float32)[hs].reshape(HDIM, 1).copy(),
                "woT": woT,
                "bo_row": bo_row,
            }
        )
    return in_maps


def _run(in_maps, trace=False):
    from concourse import bass_utils

    if "nc" not in _cache:
        _cache["nc"] = _build()
    nc = _cache["nc"]
    if trace:
        try:
            res = bass_utils.run_bass_kernel_spmd(
                nc, in_maps, core_ids=list(range(NCORES)), trace=True
            )
            return res
        except Exception:
            pass  # NTFF hook unavailable under this axon build
    try:
        res = bass_utils.run_bass_kernel_spmd(
            nc, in_maps, core_ids=list(range(NCORES)), trace=False
        )
    except Exception:
        # transient device faults (NRT_EXEC_UNIT_UNRECOVERABLE) clear on retry
        res = bass_utils.run_bass_kernel_spmd(
            nc, in_maps, core_ids=list(range(NCORES)), trace=False
        )
    return res


def kernel(x, Wq, bq, Wk, bk, Wv, bv, Wo, bo, _trace=False, _want_results=False):
    in_maps = _host_prep(x, Wq, bq, Wk, bk, Wv, bv, Wo, bo)
    res = _run(in_maps, trace=_trace)
    # core j's out rows 128w..128w+128 are global rows 1024w + 128j ..+128
    parts = np.stack(
        [np.asarray(res.results[c]["out"]).reshape(4, 128, D) for c in range(NCORES)]
    )  # [j, w, r, D]
    full = (
        parts.transpose(1, 0, 2, 3).reshape(B, T, D).astype(np.float32)
    )
    if _want_results:
        return full, res
    return full


# revision 47
# speedup vs baseline: 1.0040x; 1.0020x over previous
"""Causal self-attention on 8 trn2 NeuronCores.

Sharding: tensor-parallel over heads (2 heads/core) for QKV+attention.  Row
ownership for the output projection is window-interleaved: window w = global
rows [1024w, 1024w+1024) and core j owns rows 1024w+128j..+128 of every
window.  This makes the head-split -> row-split reshard expressible as FOUR
per-window 8-rank AllToAlls that are issued as soon as each strip-pair's
attention completes, hiding most of the collective cost under attention
compute (windows 1+2 are merged into one collective; the last window's
collective is small and constant-overhead-dominated).

Softmax normalization moves to the RECEIVE side of the collective: each core
ships unnormalized attn-out rows plus per-(head,row) reciprocal rows, and the
receiver scales the gathered [1024 dims x 128 rows] tile once before the
output projection.

All matmuls run in bf16 with f32 PSUM accumulation.  Attention is computed in
"scores transposed" layout ([keys, queries] on chip); softmax denominators
come from a ones column appended to V, and the causal mask is a
multiplicative {0,1} bf16 mask (generated on-chip via affine_select) applied
after exp (safe: scores are O(6), no overflow without max-subtraction).
"""

import numpy as np
import ml_dtypes

B, T, D, H, HD = 2, 2048, 1024, 16, 64
NCORES = 8
R = B * T              # 4096 global rows (b*T + t)
HPC = H // NCORES      # 2 heads per core
HDIM = HPC * HD        # 128 dims per core
ROWS_PER_CORE = R // NCORES  # 512
NKT = D // 128         # 8 contraction tiles
NSTRIP = T // 512      # 4 query strips per batch
WPS = 130              # rows per (window, shard) in cc buffers: 128 dims + 2 rec

_BF16 = ml_dtypes.bfloat16
_cache = {}


def _patch_tile_drain():
    """This walrus build rejects >1 sync wait on SP CTRL instructions; split
    the Tile tail-drain waits across single-wait nops."""
    import concourse.mybir as mybir
    import concourse.tile as tile_mod
    from concourse.vector_clock import ScopedClock

    if getattr(tile_mod.TileContext, "_drain_patched", False):
        return

    def _drain_and_barrier(self, tick_clock, wait_clock):
        nc = self.nc
        dummy = mybir.InstNoOp(
            name=nc.get_next_instruction_name(),
            engine=mybir.EngineType.SP,
            ins=[],
            outs=[],
        )
        wait_clock.add_sem_waits(dummy, ScopedClock({None: tick_clock.global_clock}))
        waits = list(dummy.sync_info.on_wait) if dummy.sync_info else []
        for i in range(len(waits)):
            w = nc.sync.nop(nofuse=True, hint="tail_drain_wait")
            w.ins.sync_info = mybir.SyncInfo(on_wait=waits[i : i + 1], on_update=[])
        nc.sync.drain()
        nc.all_engine_barrier()
        assert self.sems is not None
        popped = nc._tile_sem_poison_stack.pop()
        assert popped is self._sem_poison
        nc.clear_and_free_semaphores(list(self.sems.allocated().values()))
        nc.all_engine_barrier()

    tile_mod.TileContext._drain_and_barrier = _drain_and_barrier

    # Body instructions can also accumulate >2 waits (CTRL structs take 1,
    # other structs 2 on this walrus).  Before lowering, move excess waits
    # onto single-wait nops inserted just before the instruction on the same
    # engine stream.
    _orig_lower = tile_mod.TileContext._lower_ordered_insts

    def _lower_split_waits(self, ordered):
        nc = self.nc
        for bb_name, insts in ordered.items():
            new_insts = []
            for inst in insts:
                si = getattr(inst, "sync_info", None)
                waits = list(si.on_wait) if si is not None and si.on_wait else []
                limit = 1
                if len(waits) > limit and inst.engine is not None:
                    keep = waits[: limit - 1] if limit > 1 else []
                    spill = waits[len(keep) :][:-1]
                    keep = keep + [waits[-1]]
                    for w in spill:
                        nop = mybir.InstNoOp(
                            name=nc.get_next_instruction_name(),
                            engine=inst.engine,
                            ins=[],
                            outs=[],
                        )
                        nop.sync_info = mybir.SyncInfo(on_wait=[w], on_update=[])
                        nop.debug = inst.debug
                        new_insts.append(nop)
                    inst.sync_info = mybir.SyncInfo(
                        on_wait=keep, on_update=list(si.on_update or [])
                    )
                new_insts.append(inst)
            ordered[bb_name] = new_insts
        return _orig_lower(self, ordered)

    tile_mod.TileContext._lower_ordered_insts = _lower_split_waits
    tile_mod.TileContext._drain_patched = True


def _build():
    import concourse.bass as bass
    import concourse.mybir as mybir
    import concourse.tile as tile
    from concourse.tile import add_dep_helper
    from concourse.masks import make_identity

    _patch_tile_drain()
    f32 = mybir.dt.float32
    bf16 = mybir.dt.bfloat16

    nc = bass.Bass("TRN2", target_bir_lowering=False, debug=False, num_devices=NCORES)

    # ---- DRAM I/O (per core) ----
    xT = nc.dram_tensor("xT", [D, R], bf16, kind="ExternalInput").ap()
    wqT = nc.dram_tensor("wqT", [D, HDIM], bf16, kind="ExternalInput").ap()
    wkT = nc.dram_tensor("wkT", [D, HDIM], bf16, kind="ExternalInput").ap()
    wvT = nc.dram_tensor("wvT", [D, HDIM], bf16, kind="ExternalInput").ap()
    bq_s = nc.dram_tensor("bq_s", [HDIM, 1], f32, kind="ExternalInput").ap()
    bk_s = nc.dram_tensor("bk_s", [HDIM, 1], f32, kind="ExternalInput").ap()
    bv_s = nc.dram_tensor("bv_s", [HDIM, 1], f32, kind="ExternalInput").ap()
    woT = nc.dram_tensor("woT", [D, D], bf16, kind="ExternalInput").ap()
    bo_row = nc.dram_tensor("bo_row", [1, D], bf16, kind="ExternalInput").ap()
    out = nc.dram_tensor("out", [ROWS_PER_CORE, D], bf16, kind="ExternalOutput").ap()

    # collective bounce buffers. windows: w = 2b + s0//2 covers global rows
    # [1024w, 1024w+1024); shard j = core j's 128 owned rows of the window.
    # shard layout (WPS=130 rows): 0:128 = 2-head dims, 128:130 = reciprocals.
    # groups: A=[w0], B=[w1,w2] (shard = w1 130 rows then w2 130 rows), C=[w3].
    ccA_in = nc.dram_tensor("ccA_in", [NCORES * WPS, 128], bf16).ap()
    ccA_out = nc.dram_tensor("ccA_out", [NCORES * WPS, 128], bf16).ap()
    ccB_in = nc.dram_tensor("ccB_in", [NCORES * 2 * WPS, 128], bf16).ap()
    ccB_out = nc.dram_tensor("ccB_out", [NCORES * 2 * WPS, 128], bf16).ap()
    ccC_in = nc.dram_tensor("ccC_in", [NCORES * WPS, 128], bf16).ap()
    ccC_out = nc.dram_tensor("ccC_out", [NCORES * WPS, 128], bf16).ap()
    # per-window: (in_ap, out_ap, row offset inside shard, shard stride rows)
    wininfo = {
        0: (ccA_in, ccA_out, 0, WPS),
        1: (ccB_in, ccB_out, 0, 2 * WPS),
        2: (ccB_in, ccB_out, WPS, 2 * WPS),
        3: (ccC_in, ccC_out, 0, WPS),
    }
    win_cc_tag = {0: "A", 1: "B", 2: "B", 3: "C"}

    with tile.TileContext(nc) as tc:
        import contextlib
        import concourse.bass as _bass

        with contextlib.ExitStack() as ctx:
            singles = ctx.enter_context(tc.tile_pool(name="singles", bufs=1))

            # ---- weights on fast queues; x chunks fill the two HWDGE queues
            # (the gpsimd SWDGE queue pays ~1us of Pool desc-gen per DMA, so
            # it only carries the non-urgent loads: wq/biases/bo/wo).
            w_sb = {}
            for (name, src), eng in zip(
                (("v", wvT), ("q", wqT), ("k", wkT)),
                (nc.sync, nc.gpsimd, nc.scalar),
            ):
                t = singles.tile([128, NKT, HDIM], bf16, tag=f"w{name}", name=f"w{name}")
                eng.dma_start(out=t, in_=src.rearrange("(k p) c -> p k c", p=128))
                w_sb[name] = t
            bias_sb = {}
            for name, src in (("q", bq_s), ("k", bk_s), ("v", bv_s)):
                t = singles.tile([HDIM, 1], f32, tag=f"b{name}", name=f"b{name}")
                nc.gpsimd.dma_start(out=t, in_=src)
                bias_sb[name] = t
            # x in 32 chunks [128, 1024], quarter-major k-minor so quarter 0's
            # contraction tiles land first and QKV starts ~5us in.
            feed_engs = [nc.sync, nc.gpsimd, nc.scalar]
            xc_sb = [[None] * NKT for _ in range(4)]
            for qi in range(4):
                for k in range(NKT):
                    t = singles.tile([128, 1024], bf16, tag=f"xc{qi}_{k}")
                    if qi == 0:
                        # quarter 0 gates the first matmuls: keep it off the
                        # slow gpsimd SWDGE queue
                        eng = [nc.sync, nc.scalar][k % 2]
                    else:
                        eng = feed_engs[(NKT * qi + k) % 3]
                    eng.dma_start(
                        out=t,
                        in_=xT[128 * k : 128 * (k + 1), 1024 * qi : 1024 * (qi + 1)],
                    )
                    xc_sb[qi][k] = t
            bo_sb = singles.tile([1, D], bf16, tag="bo")
            nc.gpsimd.dma_start(out=bo_sb, in_=bo_row)
            wo_sb = singles.tile([128, NKT, D], bf16, tag="wo")
            nc.sync.dma_start(out=wo_sb, in_=woT.rearrange("(k p) c -> p k c", p=128))

            # ---- on-chip constants ----
            ident = singles.tile([128, 128], bf16, tag="ident")
            make_identity(nc, ident)
            ones_row = singles.tile([1, 128], bf16, tag="ones")
            nc.vector.memset(ones_row, 1.0)
            # causal masks: block m is [mask_m(512) | ones(512)];
            # mask_m[r, c] = 1.0 iff c - r - 128m >= 0.
            mask_sb = singles.tile([128, 4 * 1024], bf16, tag="mask")
            nc.gpsimd.memset(mask_sb, 1.0)
            mones = singles.tile([128, 512], bf16, tag="mones")
            nc.gpsimd.memset(mones, 1.0)
            for m in range(4):
                nc.gpsimd.affine_select(
                    out=mask_sb[:, 1024 * m : 1024 * m + 512],
                    in_=mones,
                    pattern=[[1, 512]],
                    compare_op=mybir.AluOpType.is_ge,
                    fill=0.0,
                    base=-128 * m,
                    channel_multiplier=-1,
                )
            # preload the Exp activation table off the critical path
            exp_warm = singles.tile([1, 128], bf16, tag="expwarm")
            nc.scalar.activation(
                out=exp_warm,
                in_=ones_row,
                func=mybir.ActivationFunctionType.Exp,
                scale=1.0,
            )

            # persistent activations
            qT_sb = singles.tile([128, R], bf16, tag="qT")   # rows 0-63 head A dims
            kT_sb = singles.tile([128, R], bf16, tag="kT")
            vT_sb = singles.tile([128, R], bf16, tag="vT")
            # v in [keys, dims] layout per key block kb:
            #   cols 0:64 = vA, 64 = ones, 65:129 = vB, 129 = ones
            # so lhsT for head h = cols [65h, 65h+65) = [v_h | ones]:
            # attnout at psum rows 0-63, softmax sums at row 64.
            v_ab = singles.tile([128, R // 128, 130], bf16, tag="vab")
            nc.vector.memset(v_ab[:, :, 64:65], 1.0)
            nc.vector.memset(v_ab[:, :, 129:130], 1.0)

            # ---- phase 1: projections, quarter-by-quarter so compute starts
            # as soon as the first 1MiB of x lands.  V first per quarter so
            # its transposes stay off the critical path.  Emitted in two
            # batch-halves so pair-1 attention (which only needs batch 0)
            # interleaves between them and its softmax exps hide under the
            # batch-1 projection matmuls.
            def qkv_quarters(qs):
                with (
                    tc.tile_pool(name=f"qkv_ps{qs[0]}", bufs=6, space="PSUM") as pp,
                    tc.tile_pool(name=f"tr_ps{qs[0]}", bufs=2, space="PSUM") as trp,
                ):
                    for qi in qs:
                        for name, dest in (("v", vT_sb), ("q", qT_sb), ("k", kT_sb)):
                            for blk in range(2):
                                g = 2 * qi + blk
                                ps = pp.tile([128, 512], f32, tag="qkvps",
                                             name=f"ps_{name}{g}")
                                for k in range(NKT):
                                    nc.tensor.matmul(
                                        ps,
                                        w_sb[name][:, k, :],
                                        xc_sb[qi][k][:, 512 * blk : 512 * (blk + 1)],
                                        start=(k == 0),
                                        stop=(k == NKT - 1),
                                    )
                                nc.vector.tensor_scalar_add(
                                    dest[:, 512 * g : 512 * (g + 1)], ps,
                                    bias_sb[name],
                                )
                            if name == "v":
                                # transpose this quarter's V to [keys, dims]
                                for kb in range(8 * qi, 8 * qi + 8):
                                    tp = trp.tile([128, 128], bf16, tag="trps")
                                    nc.tensor.transpose(
                                        tp, vT_sb[:, 128 * kb : 128 * (kb + 1)],
                                        ident,
                                    )
                                    nc.vector.tensor_copy(
                                        v_ab[:, kb, :].rearrange(
                                            "p (g c) -> p g c", c=65
                                        )[:, :, 0:64],
                                        tp.rearrange("p (g c) -> p g c", c=64),
                                    )

            qkv_quarters([0, 1])

            # ---- phase 3: attention per (batch, strip-pair) ----
            # strips are processed in pairs (0,1) and (2,3); for key blocks
            # visible to both strips the scores psum is [128, 1024] (cols
            # 0-511 strip s, 512-1023 strip s+1) and exp runs once over it.
            cc_writes = {0: [], 1: [], 2: [], 3: []}
            collectives = {}
            af = {}
            afn = {}
            scl = {}
            for w in range(4):
                af[w] = singles.tile(
                    [128, NCORES * 128], bf16, tag=f"af{w}", name=f"af{w}"
                )
                afn[w] = singles.tile(
                    [128, NCORES * 128], bf16, tag=f"afn{w}", name=f"afn{w}"
                )
                scl[w] = singles.tile(
                    [128, NCORES * 128], bf16, tag=f"scl{w}", name=f"scl{w}"
                )

            def emit_collective(tag, windows, cin, cout):
                cc = nc.gpsimd.collective_compute(
                    "AllToAll",
                    mybir.AluOpType.bypass,
                    ins=[cin],
                    outs=[cout],
                    replica_groups=[list(range(NCORES))],
                )
                for w in windows:
                    for wr in cc_writes[w]:
                        add_dep_helper(cc.ins, wr.ins, sync=True,
                                       reason=f"cc{tag} in ready")
                collectives[tag] = cc

            def load_af(w, eng_af, eng_scl, after=()):
                cin, cout, row_off, sstride = wininfo[w]
                cc = collectives[win_cc_tag[w]]
                handles = []
                for hh in range(2):
                    src = _bass.AP(
                        tensor=cout.tensor,
                        offset=128 * (row_off + 65 * hh),
                        ap=[[128, 64], [128 * sstride, NCORES], [1, 128]],
                    )
                    rd = eng_af.dma_start(
                        out=af[w][64 * hh : 64 * (hh + 1), :].rearrange(
                            "p (i x) -> p i x", x=128
                        ),
                        in_=src,
                    )
                    add_dep_helper(rd.ins, cc.ins, sync=True, reason="cc out ready")
                    handles.append(rd.ins)
                    ssrc = _bass.AP(
                        tensor=cout.tensor,
                        offset=128 * (row_off + 65 * hh + 64),
                        ap=[[0, 64], [128 * sstride, NCORES], [1, 128]],
                    )
                    rd = eng_scl.dma_start(
                        out=scl[w][64 * hh : 64 * (hh + 1), :].rearrange(
                            "p (i x) -> p i x", x=128
                        ),
                        in_=ssrc,
                    )
                    add_dep_helper(rd.ins, cc.ins, sync=True, reason="cc rec ready")
                    handles.append(rd.ins)
                for hnd in handles:
                    for prior in after:
                        add_dep_helper(hnd, prior, sync=False, reason="queue order")
                return handles

            with (
                tc.tile_pool(name="p_sb", bufs=10) as ppool,
                tc.tile_pool(name="att_sb", bufs=10) as apool,
                tc.tile_pool(name="rec_sb", bufs=4) as rpool,
            ):
                last_exps = []
                af_early = []

                def emit_pair(b, s0, scp, avp):
                        s1 = s0 + 1
                        w = 2 * b + s0 // 2
                        last_pair = w == 3
                        qc1 = slice(T * b + 512 * s1, T * b + 512 * (s1 + 1))
                        psV = {}
                        for h in ("A", "B"):
                            for sx in (s0, s1):
                                psV[(h, sx)] = avp.tile(
                                    [128, 512], f32, tag="av", name=f"psV_{h}{w}{sx}"
                                )
                        nkb0, nkb1 = 4 * (s0 + 1), 4 * (s1 + 1)
                        for kb in range(nkb1):
                            krange = slice(T * b + 128 * kb, T * b + 128 * (kb + 1))
                            gkb = (T // 128) * b + kb
                            both = kb < nkb0
                            p_of = {}
                            # pass 1: scores + exp + mask for BOTH heads, so
                            # PE fills with head-B scores while head-A exps
                            for hi, h in enumerate(("A", "B")):
                                rows = slice(64 * hi, 64 * (hi + 1))
                                psS = scp.tile([128, 1024], f32, tag="sc")
                                p = ppool.tile([128, 1024], bf16, tag="p")
                                p_of[h] = p
                                scale = 1.0 / float(np.sqrt(HD))
                                if both:
                                    # cols [0:off) of the s0 half are fully
                                    # causally masked -> skip them entirely
                                    m = kb - 4 * s0
                                    off = 128 * m if m >= 0 else 0
                                    nc.tensor.matmul(
                                        psS[:, off:512],
                                        kT_sb[rows, krange],
                                        qT_sb[
                                            rows,
                                            T * b + 512 * s0 + off
                                            : T * b + 512 * (s0 + 1),
                                        ],
                                        start=True,
                                        stop=True,
                                        tile_position=(64 * hi, 0),
                                    )
                                    nc.tensor.matmul(
                                        psS[:, 512:1024],
                                        kT_sb[rows, krange],
                                        qT_sb[rows, qc1],
                                        start=True,
                                        stop=True,
                                        tile_position=(64 * hi, 0),
                                    )
                                    ei = nc.scalar.activation(
                                        out=p[:, off:1024],
                                        in_=psS[:, off:1024],
                                        func=mybir.ActivationFunctionType.Exp,
                                        scale=scale,
                                    )
                                    if last_pair:
                                        last_exps.append(ei.ins)
                                    if m >= 0:
                                        # mask strip s0 half; s1 fully visible
                                        nc.vector.tensor_mul(
                                            p[:, off:1024],
                                            p[:, off:1024],
                                            mask_sb[
                                                :,
                                                1024 * m + off : 1024 * (m + 1),
                                            ],
                                        )
                                else:
                                    m = kb - 4 * s1
                                    off = 128 * m if m >= 0 else 0
                                    nc.tensor.matmul(
                                        psS[:, 512 + off : 1024],
                                        kT_sb[rows, krange],
                                        qT_sb[
                                            rows,
                                            T * b + 512 * s1 + off
                                            : T * b + 512 * (s1 + 1),
                                        ],
                                        start=True,
                                        stop=True,
                                        tile_position=(64 * hi, 0),
                                    )
                                    ei = nc.scalar.activation(
                                        out=p[:, 512 + off : 1024],
                                        in_=psS[:, 512 + off : 1024],
                                        func=mybir.ActivationFunctionType.Exp,
                                        scale=scale,
                                    )
                                    if last_pair:
                                        last_exps.append(ei.ins)
                                    if m >= 0:
                                        nc.vector.tensor_mul(
                                            p[:, 512 + off : 1024],
                                            p[:, 512 + off : 1024],
                                            mask_sb[
                                                :,
                                                1024 * m + off : 1024 * m + 512,
                                            ],
                                        )
                            # pass 2: attnV accumulate; lhsT = [v_h | ones]:
                            # attnout rows 0-63, softmax sums row 64.
                            m = kb - 4 * (s0 if both else s1)
                            off = 128 * m if m >= 0 else 0
                            for hi, h in enumerate(("A", "B")):
                                p = p_of[h]
                                lhsT = v_ab[:, gkb, 65 * hi : 65 * hi + 65]
                                if both:
                                    nc.tensor.matmul(
                                        psV[(h, s0)][0:65, off:512],
                                        lhsT,
                                        p[:, off:512],
                                        start=(kb == 0),
                                        stop=(kb == nkb0 - 1),
                                    )
                                    nc.tensor.matmul(
                                        psV[(h, s1)][0:65, 0:512],
                                        lhsT,
                                        p[:, 512:1024],
                                        start=(kb == 0),
                                        stop=(kb == nkb1 - 1),
                                    )
                                else:
                                    nc.tensor.matmul(
                                        psV[(h, s1)][0:65, off:512],
                                        lhsT,
                                        p[:, 512 + off : 1024],
                                        start=False,
                                        stop=(kb == nkb1 - 1),
                                    )
                        # ship unnormalized attn-out + reciprocal rows into
                        # this window's collective buffer.
                        cin, cout, row_off, sstride = wininfo[w]
                        for sx in (s0, s1):
                            base_j = 4 * (sx % 2)
                            if sx % 2 == 0:
                                eng = nc.sync
                            elif last_pair:
                                # ACT queue is free once the pair's exps are
                                # done; avoids serial Pool SWDGE desc-gen on
                                # the critical chain into the last collective
                                eng = nc.scalar
                            else:
                                eng = nc.gpsimd
                            # per head: [attn-out rows 0:64 | reciprocal row
                            # 64], shipped as ONE dma into the shard's 65-row
                            # head group.  Copies split across DVE/Pool on the
                            # last pair so the final drain chain is shorter.
                            for hi, h in enumerate(("A", "B")):
                                att = apool.tile([65, 512], bf16, tag="att")
                                nc.vector.tensor_copy(att[0:64, :], psV[(h, sx)][0:64, :])
                                with nc.allow_low_precision("bf16 softmax recip"):
                                    nc.vector.reciprocal(
                                        att[64:65, :], psV[(h, sx)][64:65, :]
                                    )
                                dst = _bass.AP(
                                    tensor=cin.tensor,
                                    offset=128 * (sstride * base_j + row_off + 65 * hi),
                                    ap=[[128, 65], [128 * sstride, 4], [1, 128]],
                                )
                                wr = eng.dma_start(
                                    out=dst,
                                    in_=att.rearrange("p (c x) -> p c x", x=128),
                                )
                                if last_pair and sx % 2 == 1:
                                    for e in last_exps:
                                        add_dep_helper(wr.ins, e, sync=False,
                                                       reason="act q order")
                                cc_writes[w].append(wr)
                        # issue collectives as their windows complete
                        if w == 0:
                            emit_collective("A", [0], ccA_in, ccA_out)
                            af_early.extend(
                                load_af(
                                    0, nc.sync, nc.sync,
                                    after=[wr.ins for wr in cc_writes[0]],
                                )
                            )
                        elif w == 2:
                            emit_collective("B", [1, 2], ccB_in, ccB_out)
                        elif w == 3:
                            emit_collective("C", [3], ccC_in, ccC_out)

                qkv_quarters([2, 3])
                with (
                    tc.tile_pool(name="sc_ps1", bufs=2, space="PSUM") as scp1,
                    tc.tile_pool(name="av_ps1", bufs=4, space="PSUM") as avp1,
                ):
                    emit_pair(0, 0, scp1, avp1)
                    emit_pair(0, 2, scp1, avp1)
                    emit_pair(1, 0, scp1, avp1)
                    emit_pair(1, 2, scp1, avp1)

            # ---- phase 5: output projection, one 128-row block per window.
            with (
                tc.tile_pool(name="op_ps", bufs=2, space="PSUM") as op,
                tc.tile_pool(name="out_sb", bufs=2) as opool,
            ):
                warm = op.tile([128, 512], f32, tag="op", name="warm_ps")
                out_dmas = []
                copies = []

                def compute_outproj(w):
                    nc.vector.tensor_mul(afn[w], af[w], scl[w])
                    o_sb = opool.tile([128, D], bf16, tag="osb")
                    for n in range(D // 512):
                        ps = op.tile([128, 512], f32, tag="op")
                        for i in range(NCORES):
                            nc.tensor.matmul(
                                ps,
                                afn[w][:, 128 * i : 128 * (i + 1)],
                                wo_sb[:, i, 512 * n : 512 * (n + 1)],
                                start=(i == 0),
                                stop=False,
                            )
                        nc.tensor.matmul(
                            ps,
                            ones_row,
                            bo_sb[:, 512 * n : 512 * (n + 1)],
                            start=False,
                            stop=True,
                        )
                        cp = nc.scalar.copy(o_sb[:, 512 * n : 512 * (n + 1)], ps)
                        copies.append(cp.ins)
                        if w == 3:
                            # the last window's out DMA is the critical tail:
                            # ship each half as soon as its copy lands
                            od = nc.sync.dma_start(
                                out=out[
                                    128 * w : 128 * (w + 1),
                                    512 * n : 512 * (n + 1),
                                ],
                                in_=o_sb[:, 512 * n : 512 * (n + 1)],
                            )
                            out_dmas.append(od.ins)
                    if w != 3:
                        od = [nc.sync, nc.gpsimd][w % 2].dma_start(
                            out=out[128 * w : 128 * (w + 1), :], in_=o_sb
                        )
                        out_dmas.append(od.ins)

                def pe_warm(n):
                    # keep the PE array busy/ramped while a collective drains
                    for _ in range(n):
                        nc.tensor.matmul(
                            warm, ident, kT_sb[:, 0:512], start=True, stop=True
                        )

                # windows 0-2 must not be hoisted ahead of the tail attention
                # writes they share queues with
                tail_writes = [wr.ins for wr in cc_writes[3]]
                # keep the early af0 load (emitted mid-attention) ahead of the
                # later attention DMAs on SP so outproj(0) is ready at
                # attention end
                for wr in cc_writes[2] + cc_writes[3]:
                    for hnd in af_early:
                        add_dep_helper(wr.ins, hnd, sync=False,
                                       reason="af0 before tail writes")
                compute_outproj(0)
                load_af(1, nc.sync, nc.sync, after=tail_writes)
                load_af(2, nc.sync, nc.sync, after=tail_writes)
                pe_warm(12)
                compute_outproj(1)
                compute_outproj(2)
                load_af(3, nc.sync, nc.scalar,
                        after=tail_writes + out_dmas + copies + last_exps)
                pe_warm(89)
                compute_outproj(3)

    return nc


def _host_prep(x, Wq, bq, Wk, bk, Wv, bv, Wo, bo):
    """Build the 8 per-core input maps."""
    x = np.asarray(x, np.float32)
    xT = np.ascontiguousarray(x.reshape(R, D).T).astype(_BF16)
    woT = np.ascontiguousarray(np.asarray(Wo, np.float32).T).astype(_BF16)
    bo_row = np.asarray(bo, np.float32).reshape(1, D).astype(_BF16)

    in_maps = []
    for core in range(NCORES):
        hs = slice(HDIM * core, HDIM * (core + 1))
        in_maps.append(
            {
                "xT": xT,
                "wqT": np.ascontiguousarray(np.asarray(Wq, np.float32)[hs, :].T).astype(_BF16),
                "wkT": np.ascontiguousarray(np.asarray(Wk, np.float32)[hs, :].T).astype(_BF16),
                "wvT": np.ascontiguousarray(np.asarray(Wv, np.float32)[hs, :].T).astype(_BF16),
                "bq_s": np.asarray(bq, np.float32)[hs].reshape(HDIM, 1).copy(),
                "bk_s": np.asarray(bk, np.float32)[hs].reshape(HDIM, 1).copy(),
                "bv_s": np.asarray(bv, np.float32)[hs].reshape(HDIM, 1).copy(),
                "woT": woT,
                "bo_row": bo_row,
            }
        )
    return in_maps


def _run(in_maps, trace=False):
    from concourse import bass_utils

    if "nc" not in _cache:
        _cache["nc"] = _build()
    nc = _cache["nc"]
    if trace:
        try:
            res = bass_utils.run_bass_kernel_spmd(
                nc, in_maps, core_ids=list(range(NCORES)), trace=True
            )
            return res
        except Exception:
            pass  # NTFF hook unavailable under this axon build
    try:
        res = bass_utils.run_bass_kernel_spmd(
            nc, in_maps, core_ids=list(range(NCORES)), trace=False
        )
    except Exception:
        # transient device faults (NRT_EXEC_UNIT_UNRECOVERABLE) clear on retry
        res = bass_utils.run_bass_kernel_spmd(
            nc, in_maps, core_ids=list(range(NCORES)), trace=False
        )
    return res


def kernel(x, Wq, bq, Wk, bk, Wv, bv, Wo, bo, _trace=False, _want_results=False):
    in_maps = _host_prep(x, Wq, bq, Wk, bk, Wv, bv, Wo, bo)
    res = _run(in_maps, trace=_trace)
    # core j's out rows 128w..128w+128 are global rows 1024w + 128j ..+128
    parts = np.stack(
        [np.asarray(res.results[c]["out"]).reshape(4, 128, D) for c in range(NCORES)]
    )  # [j, w, r, D]
    full = (
        parts.transpose(1, 0, 2, 3).reshape(B, T, D).astype(np.float32)
    )
    if _want_results:
        return full, res
    return full


# revision 48
# speedup vs baseline: 1.0045x; 1.0005x over previous
"""Causal self-attention on 8 trn2 NeuronCores.

Sharding: tensor-parallel over heads (2 heads/core) for QKV+attention.  Row
ownership for the output projection is window-interleaved: window w = global
rows [1024w, 1024w+1024) and core j owns rows 1024w+128j..+128 of every
window.  This makes the head-split -> row-split reshard expressible as FOUR
per-window 8-rank AllToAlls that are issued as soon as each strip-pair's
attention completes, hiding most of the collective cost under attention
compute (windows 1+2 are merged into one collective; the last window's
collective is small and constant-overhead-dominated).

Softmax normalization moves to the RECEIVE side of the collective: each core
ships unnormalized attn-out rows plus per-(head,row) reciprocal rows, and the
receiver scales the gathered [1024 dims x 128 rows] tile once before the
output projection.

All matmuls run in bf16 with f32 PSUM accumulation.  Attention is computed in
"scores transposed" layout ([keys, queries] on chip); softmax denominators
come from a ones column appended to V, and the causal mask is a
multiplicative {0,1} bf16 mask (generated on-chip via affine_select) applied
after exp (safe: scores are O(6), no overflow without max-subtraction).
"""

import numpy as np
import ml_dtypes

B, T, D, H, HD = 2, 2048, 1024, 16, 64
NCORES = 8
R = B * T              # 4096 global rows (b*T + t)
HPC = H // NCORES      # 2 heads per core
HDIM = HPC * HD        # 128 dims per core
ROWS_PER_CORE = R // NCORES  # 512
NKT = D // 128         # 8 contraction tiles
NSTRIP = T // 512      # 4 query strips per batch
WPS = 130              # rows per (window, shard) in cc buffers: 128 dims + 2 rec

_BF16 = ml_dtypes.bfloat16
_cache = {}


def _patch_tile_drain():
    """This walrus build rejects >1 sync wait on SP CTRL instructions; split
    the Tile tail-drain waits across single-wait nops."""
    import concourse.mybir as mybir
    import concourse.tile as tile_mod
    from concourse.vector_clock import ScopedClock

    if getattr(tile_mod.TileContext, "_drain_patched", False):
        return

    def _drain_and_barrier(self, tick_clock, wait_clock):
        nc = self.nc
        dummy = mybir.InstNoOp(
            name=nc.get_next_instruction_name(),
            engine=mybir.EngineType.SP,
            ins=[],
            outs=[],
        )
        wait_clock.add_sem_waits(dummy, ScopedClock({None: tick_clock.global_clock}))
        waits = list(dummy.sync_info.on_wait) if dummy.sync_info else []
        for i in range(len(waits)):
            w = nc.sync.nop(nofuse=True, hint="tail_drain_wait")
            w.ins.sync_info = mybir.SyncInfo(on_wait=waits[i : i + 1], on_update=[])
        nc.sync.drain()
        nc.all_engine_barrier()
        assert self.sems is not None
        popped = nc._tile_sem_poison_stack.pop()
        assert popped is self._sem_poison
        nc.clear_and_free_semaphores(list(self.sems.allocated().values()))
        nc.all_engine_barrier()

    tile_mod.TileContext._drain_and_barrier = _drain_and_barrier

    # Body instructions can also accumulate >2 waits (CTRL structs take 1,
    # other structs 2 on this walrus).  Before lowering, move excess waits
    # onto single-wait nops inserted just before the instruction on the same
    # engine stream.
    _orig_lower = tile_mod.TileContext._lower_ordered_insts

    def _lower_split_waits(self, ordered):
        nc = self.nc
        for bb_name, insts in ordered.items():
            new_insts = []
            for inst in insts:
                si = getattr(inst, "sync_info", None)
                waits = list(si.on_wait) if si is not None and si.on_wait else []
                limit = 1
                if len(waits) > limit and inst.engine is not None:
                    keep = waits[: limit - 1] if limit > 1 else []
                    spill = waits[len(keep) :][:-1]
                    keep = keep + [waits[-1]]
                    for w in spill:
                        nop = mybir.InstNoOp(
                            name=nc.get_next_instruction_name(),
                            engine=inst.engine,
                            ins=[],
                            outs=[],
                        )
                        nop.sync_info = mybir.SyncInfo(on_wait=[w], on_update=[])
                        nop.debug = inst.debug
                        new_insts.append(nop)
                    inst.sync_info = mybir.SyncInfo(
                        on_wait=keep, on_update=list(si.on_update or [])
                    )
                new_insts.append(inst)
            ordered[bb_name] = new_insts
        return _orig_lower(self, ordered)

    tile_mod.TileContext._lower_ordered_insts = _lower_split_waits
    tile_mod.TileContext._drain_patched = True


def _build():
    import concourse.bass as bass
    import concourse.mybir as mybir
    import concourse.tile as tile
    from concourse.tile import add_dep_helper
    from concourse.masks import make_identity

    _patch_tile_drain()
    f32 = mybir.dt.float32
    bf16 = mybir.dt.bfloat16

    nc = bass.Bass("TRN2", target_bir_lowering=False, debug=False, num_devices=NCORES)

    # ---- DRAM I/O (per core) ----
    xT = nc.dram_tensor("xT", [D, R], bf16, kind="ExternalInput").ap()
    wqT = nc.dram_tensor("wqT", [D, HDIM], bf16, kind="ExternalInput").ap()
    wkT = nc.dram_tensor("wkT", [D, HDIM], bf16, kind="ExternalInput").ap()
    wvT = nc.dram_tensor("wvT", [D, HDIM], bf16, kind="ExternalInput").ap()
    bq_s = nc.dram_tensor("bq_s", [HDIM, 1], f32, kind="ExternalInput").ap()
    bk_s = nc.dram_tensor("bk_s", [HDIM, 1], f32, kind="ExternalInput").ap()
    bv_s = nc.dram_tensor("bv_s", [HDIM, 1], f32, kind="ExternalInput").ap()
    woT = nc.dram_tensor("woT", [D, D], bf16, kind="ExternalInput").ap()
    bo_row = nc.dram_tensor("bo_row", [1, D], bf16, kind="ExternalInput").ap()
    out = nc.dram_tensor("out", [ROWS_PER_CORE, D], bf16, kind="ExternalOutput").ap()

    # collective bounce buffers. windows: w = 2b + s0//2 covers global rows
    # [1024w, 1024w+1024); shard j = core j's 128 owned rows of the window.
    # shard layout (WPS=130 rows): 0:128 = 2-head dims, 128:130 = reciprocals.
    # groups: A=[w0], B=[w1,w2] (shard = w1 130 rows then w2 130 rows), C=[w3].
    ccA_in = nc.dram_tensor("ccA_in", [NCORES * WPS, 128], bf16).ap()
    ccA_out = nc.dram_tensor("ccA_out", [NCORES * WPS, 128], bf16).ap()
    ccB_in = nc.dram_tensor("ccB_in", [NCORES * 2 * WPS, 128], bf16).ap()
    ccB_out = nc.dram_tensor("ccB_out", [NCORES * 2 * WPS, 128], bf16).ap()
    ccC_in = nc.dram_tensor("ccC_in", [NCORES * WPS, 128], bf16).ap()
    ccC_out = nc.dram_tensor("ccC_out", [NCORES * WPS, 128], bf16).ap()
    # per-window: (in_ap, out_ap, row offset inside shard, shard stride rows)
    wininfo = {
        0: (ccA_in, ccA_out, 0, WPS),
        1: (ccB_in, ccB_out, 0, 2 * WPS),
        2: (ccB_in, ccB_out, WPS, 2 * WPS),
        3: (ccC_in, ccC_out, 0, WPS),
    }
    win_cc_tag = {0: "A", 1: "B", 2: "B", 3: "C"}

    with tile.TileContext(nc) as tc:
        import contextlib
        import concourse.bass as _bass

        with contextlib.ExitStack() as ctx:
            singles = ctx.enter_context(tc.tile_pool(name="singles", bufs=1))

            # ---- weights on fast queues; x chunks fill the two HWDGE queues
            # (the gpsimd SWDGE queue pays ~1us of Pool desc-gen per DMA, so
            # it only carries the non-urgent loads: wq/biases/bo/wo).
            w_sb = {}
            for (name, src), eng in zip(
                (("v", wvT), ("q", wqT), ("k", wkT)),
                (nc.sync, nc.gpsimd, nc.scalar),
            ):
                t = singles.tile([128, NKT, HDIM], bf16, tag=f"w{name}", name=f"w{name}")
                eng.dma_start(out=t, in_=src.rearrange("(k p) c -> p k c", p=128))
                w_sb[name] = t
            bias_sb = {}
            for name, src in (("q", bq_s), ("k", bk_s), ("v", bv_s)):
                t = singles.tile([HDIM, 1], f32, tag=f"b{name}", name=f"b{name}")
                nc.gpsimd.dma_start(out=t, in_=src)
                bias_sb[name] = t
            # x in 32 chunks [128, 1024], quarter-major k-minor so quarter 0's
            # contraction tiles land first and QKV starts ~5us in.
            feed_engs = [nc.sync, nc.gpsimd, nc.scalar]
            xc_sb = [[None] * NKT for _ in range(4)]
            for qi in range(4):
                for k in range(NKT):
                    t = singles.tile([128, 1024], bf16, tag=f"xc{qi}_{k}")
                    if qi == 0:
                        # quarter 0 gates the first matmuls: keep it off the
                        # slow gpsimd SWDGE queue
                        eng = [nc.sync, nc.scalar][k % 2]
                    else:
                        eng = feed_engs[(NKT * qi + k) % 3]
                    eng.dma_start(
                        out=t,
                        in_=xT[128 * k : 128 * (k + 1), 1024 * qi : 1024 * (qi + 1)],
                    )
                    xc_sb[qi][k] = t
            bo_sb = singles.tile([1, D], bf16, tag="bo")
            nc.gpsimd.dma_start(out=bo_sb, in_=bo_row)
            wo_sb = singles.tile([128, NKT, D], bf16, tag="wo")
            nc.sync.dma_start(out=wo_sb, in_=woT.rearrange("(k p) c -> p k c", p=128))

            # ---- on-chip constants ----
            ident = singles.tile([128, 128], bf16, tag="ident")
            make_identity(nc, ident)
            ones_row = singles.tile([1, 128], bf16, tag="ones")
            nc.vector.memset(ones_row, 1.0)
            # causal masks: block m is [mask_m(512) | ones(512)];
            # mask_m[r, c] = 1.0 iff c - r - 128m >= 0.
            mask_sb = singles.tile([128, 4 * 1024], bf16, tag="mask")
            nc.gpsimd.memset(mask_sb, 1.0)
            mones = singles.tile([128, 512], bf16, tag="mones")
            nc.gpsimd.memset(mones, 1.0)
            for m in range(4):
                nc.gpsimd.affine_select(
                    out=mask_sb[:, 1024 * m : 1024 * m + 512],
                    in_=mones,
                    pattern=[[1, 512]],
                    compare_op=mybir.AluOpType.is_ge,
                    fill=0.0,
                    base=-128 * m,
                    channel_multiplier=-1,
                )
            # preload the Exp activation table off the critical path
            exp_warm = singles.tile([1, 128], bf16, tag="expwarm")
            nc.scalar.activation(
                out=exp_warm,
                in_=ones_row,
                func=mybir.ActivationFunctionType.Exp,
                scale=1.0,
            )

            # persistent activations
            qT_sb = singles.tile([128, R], bf16, tag="qT")   # rows 0-63 head A dims
            kT_sb = singles.tile([128, R], bf16, tag="kT")
            vT_sb = singles.tile([128, R], bf16, tag="vT")
            # v in [keys, dims] layout per key block kb:
            #   cols 0:64 = vA, 64 = ones, 65:129 = vB, 129 = ones
            # so lhsT for head h = cols [65h, 65h+65) = [v_h | ones]:
            # attnout at psum rows 0-63, softmax sums at row 64.
            v_ab = singles.tile([128, R // 128, 130], bf16, tag="vab")
            nc.vector.memset(v_ab[:, :, 64:65], 1.0)
            nc.vector.memset(v_ab[:, :, 129:130], 1.0)

            # ---- phase 1: projections, quarter-by-quarter so compute starts
            # as soon as the first 1MiB of x lands.  V first per quarter so
            # its transposes stay off the critical path.  Emitted in two
            # batch-halves so pair-1 attention (which only needs batch 0)
            # interleaves between them and its softmax exps hide under the
            # batch-1 projection matmuls.
            def qkv_quarters(qs):
                with (
                    tc.tile_pool(name=f"qkv_ps{qs[0]}", bufs=6, space="PSUM") as pp,
                    tc.tile_pool(name=f"tr_ps{qs[0]}", bufs=2, space="PSUM") as trp,
                ):
                    for qi in qs:
                        for name, dest in (("v", vT_sb), ("q", qT_sb), ("k", kT_sb)):
                            for blk in range(2):
                                g = 2 * qi + blk
                                ps = pp.tile([128, 512], f32, tag="qkvps",
                                             name=f"ps_{name}{g}")
                                for k in range(NKT):
                                    nc.tensor.matmul(
                                        ps,
                                        w_sb[name][:, k, :],
                                        xc_sb[qi][k][:, 512 * blk : 512 * (blk + 1)],
                                        start=(k == 0),
                                        stop=(k == NKT - 1),
                                    )
                                nc.vector.tensor_scalar_add(
                                    dest[:, 512 * g : 512 * (g + 1)], ps,
                                    bias_sb[name],
                                )
                            if name == "v":
                                # transpose this quarter's V to [keys, dims]
                                for kb in range(8 * qi, 8 * qi + 8):
                                    tp = trp.tile([128, 128], bf16, tag="trps")
                                    nc.tensor.transpose(
                                        tp, vT_sb[:, 128 * kb : 128 * (kb + 1)],
                                        ident,
                                    )
                                    nc.vector.tensor_copy(
                                        v_ab[:, kb, :].rearrange(
                                            "p (g c) -> p g c", c=65
                                        )[:, :, 0:64],
                                        tp.rearrange("p (g c) -> p g c", c=64),
                                    )

            qkv_quarters([0, 1])

            # ---- phase 3: attention per (batch, strip-pair) ----
            # strips are processed in pairs (0,1) and (2,3); for key blocks
            # visible to both strips the scores psum is [128, 1024] (cols
            # 0-511 strip s, 512-1023 strip s+1) and exp runs once over it.
            cc_writes = {0: [], 1: [], 2: [], 3: []}
            collectives = {}
            af = {}
            afn = {}
            scl = {}
            for w in range(4):
                af[w] = singles.tile(
                    [128, NCORES * 128], bf16, tag=f"af{w}", name=f"af{w}"
                )
                afn[w] = singles.tile(
                    [128, NCORES * 128], bf16, tag=f"afn{w}", name=f"afn{w}"
                )
                scl[w] = singles.tile(
                    [128, NCORES * 128], bf16, tag=f"scl{w}", name=f"scl{w}"
                )

            def emit_collective(tag, windows, cin, cout):
                cc = nc.gpsimd.collective_compute(
                    "AllToAll",
                    mybir.AluOpType.bypass,
                    ins=[cin],
                    outs=[cout],
                    replica_groups=[list(range(NCORES))],
                )
                for w in windows:
                    for wr in cc_writes[w]:
                        add_dep_helper(cc.ins, wr.ins, sync=True,
                                       reason=f"cc{tag} in ready")
                collectives[tag] = cc

            def load_af(w, eng_af, eng_scl, after=()):
                cin, cout, row_off, sstride = wininfo[w]
                cc = collectives[win_cc_tag[w]]
                handles = []
                for hh in range(2):
                    src = _bass.AP(
                        tensor=cout.tensor,
                        offset=128 * (row_off + 65 * hh),
                        ap=[[128, 64], [128 * sstride, NCORES], [1, 128]],
                    )
                    rd = eng_af.dma_start(
                        out=af[w][64 * hh : 64 * (hh + 1), :].rearrange(
                            "p (i x) -> p i x", x=128
                        ),
                        in_=src,
                    )
                    add_dep_helper(rd.ins, cc.ins, sync=True, reason="cc out ready")
                    handles.append(rd.ins)
                    ssrc = _bass.AP(
                        tensor=cout.tensor,
                        offset=128 * (row_off + 65 * hh + 64),
                        ap=[[0, 64], [128 * sstride, NCORES], [1, 128]],
                    )
                    rd = eng_scl.dma_start(
                        out=scl[w][64 * hh : 64 * (hh + 1), :].rearrange(
                            "p (i x) -> p i x", x=128
                        ),
                        in_=ssrc,
                    )
                    add_dep_helper(rd.ins, cc.ins, sync=True, reason="cc rec ready")
                    handles.append(rd.ins)
                for hnd in handles:
                    for prior in after:
                        add_dep_helper(hnd, prior, sync=False, reason="queue order")
                return handles

            with (
                tc.tile_pool(name="p_sb", bufs=14) as ppool,
                tc.tile_pool(name="att_sb", bufs=12) as apool,
                tc.tile_pool(name="rec_sb", bufs=4) as rpool,
            ):
                last_exps = []
                af_early = []

                def emit_pair(b, s0, scp, avp):
                        s1 = s0 + 1
                        w = 2 * b + s0 // 2
                        last_pair = w == 3
                        qc1 = slice(T * b + 512 * s1, T * b + 512 * (s1 + 1))
                        psV = {}
                        for h in ("A", "B"):
                            for sx in (s0, s1):
                                psV[(h, sx)] = avp.tile(
                                    [128, 512], f32, tag="av", name=f"psV_{h}{w}{sx}"
                                )
                        nkb0, nkb1 = 4 * (s0 + 1), 4 * (s1 + 1)
                        for kb in range(nkb1):
                            krange = slice(T * b + 128 * kb, T * b + 128 * (kb + 1))
                            gkb = (T // 128) * b + kb
                            both = kb < nkb0
                            p_of = {}
                            # pass 1: scores + exp + mask for BOTH heads, so
                            # PE fills with head-B scores while head-A exps
                            for hi, h in enumerate(("A", "B")):
                                rows = slice(64 * hi, 64 * (hi + 1))
                                psS = scp.tile([128, 1024], f32, tag="sc")
                                p = ppool.tile([128, 1024], bf16, tag="p")
                                p_of[h] = p
                                scale = 1.0 / float(np.sqrt(HD))
                                if both:
                                    # cols [0:off) of the s0 half are fully
                                    # causally masked -> skip them entirely
                                    m = kb - 4 * s0
                                    off = 128 * m if m >= 0 else 0
                                    nc.tensor.matmul(
                                        psS[:, off:512],
                                        kT_sb[rows, krange],
                                        qT_sb[
                                            rows,
                                            T * b + 512 * s0 + off
                                            : T * b + 512 * (s0 + 1),
                                        ],
                                        start=True,
                                        stop=True,
                                        tile_position=(64 * hi, 0),
                                    )
                                    nc.tensor.matmul(
                                        psS[:, 512:1024],
                                        kT_sb[rows, krange],
                                        qT_sb[rows, qc1],
                                        start=True,
                                        stop=True,
                                        tile_position=(64 * hi, 0),
                                    )
                                    ei = nc.scalar.activation(
                                        out=p[:, off:1024],
                                        in_=psS[:, off:1024],
                                        func=mybir.ActivationFunctionType.Exp,
                                        scale=scale,
                                    )
                                    if last_pair:
                                        last_exps.append(ei.ins)
                                    if m >= 0:
                                        # mask strip s0 half; s1 fully visible
                                        nc.vector.tensor_mul(
                                            p[:, off:1024],
                                            p[:, off:1024],
                                            mask_sb[
                                                :,
                                                1024 * m + off : 1024 * (m + 1),
                                            ],
                                        )
                                else:
                                    m = kb - 4 * s1
                                    off = 128 * m if m >= 0 else 0
                                    nc.tensor.matmul(
                                        psS[:, 512 + off : 1024],
                                        kT_sb[rows, krange],
                                        qT_sb[
                                            rows,
                                            T * b + 512 * s1 + off
                                            : T * b + 512 * (s1 + 1),
                                        ],
                                        start=True,
                                        stop=True,
                                        tile_position=(64 * hi, 0),
                                    )
                                    ei = nc.scalar.activation(
                                        out=p[:, 512 + off : 1024],
                                        in_=psS[:, 512 + off : 1024],
                                        func=mybir.ActivationFunctionType.Exp,
                                        scale=scale,
                                    )
                                    if last_pair:
                                        last_exps.append(ei.ins)
                                    if m >= 0:
                                        nc.vector.tensor_mul(
                                            p[:, 512 + off : 1024],
                                            p[:, 512 + off : 1024],
                                            mask_sb[
                                                :,
                                                1024 * m + off : 1024 * m + 512,
                                            ],
                                        )
                            # pass 2: attnV accumulate; lhsT = [v_h | ones]:
                            # attnout rows 0-63, softmax sums row 64.
                            m = kb - 4 * (s0 if both else s1)
                            off = 128 * m if m >= 0 else 0
                            for hi, h in enumerate(("A", "B")):
                                p = p_of[h]
                                lhsT = v_ab[:, gkb, 65 * hi : 65 * hi + 65]
                                if both:
                                    nc.tensor.matmul(
                                        psV[(h, s0)][0:65, off:512],
                                        lhsT,
                                        p[:, off:512],
                                        start=(kb == 0),
                                        stop=(kb == nkb0 - 1),
                                    )
                                    nc.tensor.matmul(
                                        psV[(h, s1)][0:65, 0:512],
                                        lhsT,
                                        p[:, 512:1024],
                                        start=(kb == 0),
                                        stop=(kb == nkb1 - 1),
                                    )
                                else:
                                    nc.tensor.matmul(
                                        psV[(h, s1)][0:65, off:512],
                                        lhsT,
                                        p[:, 512 + off : 1024],
                                        start=False,
                                        stop=(kb == nkb1 - 1),
                                    )
                        # ship unnormalized attn-out + reciprocal rows into
                        # this window's collective buffer.
                        cin, cout, row_off, sstride = wininfo[w]
                        for sx in (s0, s1):
                            base_j = 4 * (sx % 2)
                            if sx % 2 == 0:
                                eng = nc.sync
                            elif last_pair:
                                # ACT queue is free once the pair's exps are
                                # done; avoids serial Pool SWDGE desc-gen on
                                # the critical chain into the last collective
                                eng = nc.scalar
                            else:
                                eng = nc.gpsimd
                            # per head: [attn-out rows 0:64 | reciprocal row
                            # 64], shipped as ONE dma into the shard's 65-row
                            # head group.  Copies split across DVE/Pool on the
                            # last pair so the final drain chain is shorter.
                            for hi, h in enumerate(("A", "B")):
                                att = apool.tile([65, 512], bf16, tag="att")
                                nc.vector.tensor_copy(att[0:64, :], psV[(h, sx)][0:64, :])
                                with nc.allow_low_precision("bf16 softmax recip"):
                                    nc.vector.reciprocal(
                                        att[64:65, :], psV[(h, sx)][64:65, :]
                                    )
                                dst = _bass.AP(
                                    tensor=cin.tensor,
                                    offset=128 * (sstride * base_j + row_off + 65 * hi),
                                    ap=[[128, 65], [128 * sstride, 4], [1, 128]],
                                )
                                wr = eng.dma_start(
                                    out=dst,
                                    in_=att.rearrange("p (c x) -> p c x", x=128),
                                )
                                if last_pair and sx % 2 == 1:
                                    for e in last_exps:
                                        add_dep_helper(wr.ins, e, sync=False,
                                                       reason="act q order")
                                cc_writes[w].append(wr)
                        # issue collectives as their windows complete
                        if w == 0:
                            emit_collective("A", [0], ccA_in, ccA_out)
                            af_early.extend(
                                load_af(
                                    0, nc.sync, nc.sync,
                                    after=[wr.ins for wr in cc_writes[0]],
                                )
                            )
                        elif w == 2:
                            emit_collective("B", [1, 2], ccB_in, ccB_out)
                        elif w == 3:
                            emit_collective("C", [3], ccC_in, ccC_out)

                qkv_quarters([2, 3])
                with (
                    tc.tile_pool(name="sc_ps1", bufs=2, space="PSUM") as scp1,
                    tc.tile_pool(name="av_ps1", bufs=4, space="PSUM") as avp1,
                ):
                    emit_pair(0, 0, scp1, avp1)
                    emit_pair(0, 2, scp1, avp1)
                    emit_pair(1, 0, scp1, avp1)
                    emit_pair(1, 2, scp1, avp1)

            # ---- phase 5: output projection, one 128-row block per window.
            with (
                tc.tile_pool(name="op_ps", bufs=2, space="PSUM") as op,
                tc.tile_pool(name="out_sb", bufs=2) as opool,
            ):
                warm = op.tile([128, 512], f32, tag="op", name="warm_ps")
                out_dmas = []
                copies = []

                def compute_outproj(w):
                    nc.vector.tensor_mul(afn[w], af[w], scl[w])
                    o_sb = opool.tile([128, D], bf16, tag="osb")
                    for n in range(D // 512):
                        ps = op.tile([128, 512], f32, tag="op")
                        for i in range(NCORES):
                            nc.tensor.matmul(
                                ps,
                                afn[w][:, 128 * i : 128 * (i + 1)],
                                wo_sb[:, i, 512 * n : 512 * (n + 1)],
                                start=(i == 0),
                                stop=False,
                            )
                        nc.tensor.matmul(
                            ps,
                            ones_row,
                            bo_sb[:, 512 * n : 512 * (n + 1)],
                            start=False,
                            stop=True,
                        )
                        cp = nc.scalar.copy(o_sb[:, 512 * n : 512 * (n + 1)], ps)
                        copies.append(cp.ins)
                        if w == 3:
                            # the last window's out DMA is the critical tail:
                            # ship each half as soon as its copy lands
                            od = nc.sync.dma_start(
                                out=out[
                                    128 * w : 128 * (w + 1),
                                    512 * n : 512 * (n + 1),
                                ],
                                in_=o_sb[:, 512 * n : 512 * (n + 1)],
                            )
                            out_dmas.append(od.ins)
                    if w != 3:
                        od = [nc.sync, nc.gpsimd][w % 2].dma_start(
                            out=out[128 * w : 128 * (w + 1), :], in_=o_sb
                        )
                        out_dmas.append(od.ins)

                def pe_warm(n):
                    # keep the PE array busy/ramped while a collective drains
                    for _ in range(n):
                        nc.tensor.matmul(
                            warm, ident, kT_sb[:, 0:512], start=True, stop=True
                        )

                # windows 0-2 must not be hoisted ahead of the tail attention
                # writes they share queues with
                tail_writes = [wr.ins for wr in cc_writes[3]]
                # keep the early af0 load (emitted mid-attention) ahead of the
                # later attention DMAs on SP so outproj(0) is ready at
                # attention end
                for wr in cc_writes[2] + cc_writes[3]:
                    for hnd in af_early:
                        add_dep_helper(wr.ins, hnd, sync=False,
                                       reason="af0 before tail writes")
                compute_outproj(0)
                load_af(1, nc.sync, nc.sync, after=tail_writes)
                load_af(2, nc.sync, nc.sync, after=tail_writes)
                pe_warm(12)
                compute_outproj(1)
                compute_outproj(2)
                load_af(3, nc.sync, nc.scalar,
                        after=tail_writes + out_dmas + copies + last_exps)
                pe_warm(89)
                compute_outproj(3)

    return nc


def _host_prep(x, Wq, bq, Wk, bk, Wv, bv, Wo, bo):
    """Build the 8 per-core input maps."""
    x = np.asarray(x, np.float32)
    xT = np.ascontiguousarray(x.reshape(R, D).T).astype(_BF16)
    woT = np.ascontiguousarray(np.asarray(Wo, np.float32).T).astype(_BF16)
    bo_row = np.asarray(bo, np.float32).reshape(1, D).astype(_BF16)

    in_maps = []
    for core in range(NCORES):
        hs = slice(HDIM * core, HDIM * (core + 1))
        in_maps.append(
            {
                "xT": xT,
                "wqT": np.ascontiguousarray(np.asarray(Wq, np.float32)[hs, :].T).astype(_BF16),
                "wkT": np.ascontiguousarray(np.asarray(Wk, np.float32)[hs, :].T).astype(_BF16),
                "wvT": np.ascontiguousarray(np.asarray(Wv, np.float32)[hs, :].T).astype(_BF16),
                "bq_s": np.asarray(bq, np.float32)[hs].reshape(HDIM, 1).copy(),
                "bk_s": np.asarray(bk, np.float32)[hs].reshape(HDIM, 1).copy(),
                "bv_s": np.asarray(bv, np.float32)[hs].reshape(HDIM, 1).copy(),
                "woT": woT,
                "bo_row": bo_row,
            }
        )
    return in_maps


def _run(in_maps, trace=False):
    from concourse import bass_utils

    if "nc" not in _cache:
        _cache["nc"] = _build()
    nc = _cache["nc"]
    if trace:
        try:
            res = bass_utils.run_bass_kernel_spmd(
                nc, in_maps, core_ids=list(range(NCORES)), trace=True
            )
            return res
        except Exception:
            pass  # NTFF hook unavailable under this axon build
    try:
        res = bass_utils.run_bass_kernel_spmd(
            nc, in_maps, core_ids=list(range(NCORES)), trace=False
        )
    except Exception:
        # transient device faults (NRT_EXEC_UNIT_UNRECOVERABLE) clear on retry
        res = bass_utils.run_bass_kernel_spmd(
            nc, in_maps, core_ids=list(range(NCORES)), trace=False
        )
    return res


def kernel(x, Wq, bq, Wk, bk, Wv, bv, Wo, bo, _trace=False, _want_results=False):
    in_maps = _host_prep(x, Wq, bq, Wk, bk, Wv, bv, Wo, bo)
    res = _run(in_maps, trace=_trace)
    # core j's out rows 128w..128w+128 are global rows 1024w + 128j ..+128
    parts = np.stack(
        [np.asarray(res.results[c]["out"]).reshape(4, 128, D) for c in range(NCORES)]
    )  # [j, w, r, D]
    full = (
        parts.transpose(1, 0, 2, 3).reshape(B, T, D).astype(np.float32)
    )
    if _want_results:
        return full, res
    return full
